# revision 8
# baseline (speedup 1.0000x reference)
"""Multi-head self-attention (B=4, T=2048, E=1024, H=16) on 8 trn2 NeuronCores.

Sharding: core (b, h) = batch b, token-half h. Each core computes K/V for the
full sequence (duplicated within the batch pair), Q for its own 8 query blocks
of 128 tokens, causal attention for those blocks, then the output projection
and LayerNorm for its own tokens.

Attention restructure (vs the 128-wide-per-head-pair baseline): each core's
query blocks are ordered by DESCENDING padded causal length (16,14,12,10 |
8,6,4,2 key blocks), so for key block j the active query blocks form a
contiguous prefix. Scores/AV run one matmul per (head, group-of-4-q-blocks,
key block) with free dim up to 512, cutting PE instruction count ~3x. The
softmax denominator division runs once per (head, group) on 512 columns.
Projection+LN for the first 4 token blocks is interleaved into the second
(light) attention group to shrink the tail.

Causal balance: query blocks are paired (j, 15-j) so both cores of a batch
process blocks with padded key-lengths 2,4,...,16; host-supplied mask tiles
encode the true causal structure, keeping the compiled program identical
across cores (SPMD).

All matmuls run in bf16 with fp32 PSUM accumulation (validated ~2e-3
scale-relative error vs the fp32 reference).
"""
import json
import numpy as np
import ml_dtypes
from contextlib import ExitStack

import concourse.bass as bass
import concourse.bass_utils as _bass_utils
import concourse.tile as tile
from concourse import mybir
from concourse.bass_utils import run_bass_kernel_spmd

# ----------------------------------------------------------------------------
# Toolchain workarounds for this container's walrus build (see birfix notes):
# 1. EVENT_SEMAPHORE_RANGE_CLEAR InstISA is rejected ("ISA wrong length").
# 2. Engine instructions only carry one semaphore-wait slot; extra waits are
#    peeled onto NoOp carriers on the same engine (order-preserving).
# ----------------------------------------------------------------------------


def _patched_clear_and_free_semaphores(self, sems):
    if not sems:
        return
    sem_nums = [s.num if hasattr(s, "num") else s for s in sems]
    self._state.prepend_free_semaphores(sem_nums)
    for poison_set in self._tile_sem_poison_stack:
        poison_set.update(sem_nums)


def _fix_bir_waits(bir_json: bytes) -> bytes:
    bir = json.loads(bir_json)
    ctr = 0
    changed = False
    for func in bir.get("functions", []):
        for blk in func.get("blocks", []):
            out = []
            for inst in blk.get("instructions", []):
                si = inst.get("sync_info") or {}
                waits = si.get("on_wait") or []
                if len(waits) > 1:
                    for w in waits[:-1]:
                        ctr += 1
                        out.append(
                            {
                                "debug": inst.get("debug"),
                                "engine": inst.get("engine", "SP"),
                                "ins": [],
                                "name": f"IWF-{ctr}",
                                "opcode": "NoOp",
                                "outs": [],
                                "sync_info": {"on_wait": [w]},
                            }
                        )
                    si = dict(si)
                    si["on_wait"] = waits[-1:]
                    inst = dict(inst)
                    inst["sync_info"] = si
                    changed = True
                out.append(inst)
            blk["instructions"] = out
    return json.dumps(bir).encode() if changed else bir_json


_orig_compile_bir_kernel = _bass_utils.compile_bir_kernel


def _patched_compile_bir_kernel(bir_json, tmpdir, neff_name="file.neff"):
    if isinstance(bir_json, str):
        bir_json = bir_json.encode()
    return _orig_compile_bir_kernel(_fix_bir_waits(bir_json), tmpdir, neff_name)


def _install_patches():
    if getattr(bass.Bass, "_mhsa_patched", False):
        return
    bass.Bass.clear_and_free_semaphores = _patched_clear_and_free_semaphores
    bass.Bass._mhsa_patched = True
    _bass_utils.compile_bir_kernel = _patched_compile_bir_kernel
    try:
        import concourse.bass2jax as _b2j

        _b2j.compile_bir_kernel = _patched_compile_bir_kernel
    except ImportError:
        pass


_install_patches()

# ----------------------------------------------------------------------------
# Problem constants (hardcoded per spec)
# ----------------------------------------------------------------------------
B, T, E, H = 4, 2048, 1024, 16
HD = E // H  # 64
P = 128
NB = T // P  # 16 query/key blocks
NQ = 8  # query blocks per core
EC = E // P  # 8 e-chunks
SCALE = 1.0 / float(np.sqrt(T))
EPS = 1e-6
BF = mybir.dt.bfloat16
F32 = mybir.dt.float32
NPBF = ml_dtypes.bfloat16

# query-block assignment: pairs (j, 15-j) so both cores of a batch pair see
# padded lengths {2,4,...,16}; blocks listed in ASCENDING padded length
BLOCKS_A = [0, 2, 4, 6, 9, 11, 13, 15]  # true lengths 1,3,5,7,10,12,14,16
BLOCKS_B = [1, 3, 5, 7, 8, 10, 12, 14]  # true lengths 2,4,6,8,9,11,13,15

# device-side q-block order: DESCENDING padded length; two groups of 4
GLS = {0: (16, 14, 12, 10), 1: (8, 6, 4, 2)}
# mask instances: (group, key block j, boundary q-block index bi); the
# boundary block is always the LAST active block of the prefix at that j
MASK_INST = []
for _g in (0, 1):
    for _j in range(GLS[_g][0]):
        for _bi, _L in enumerate(GLS[_g]):
            if _j in (_L - 2, _L - 1):
                MASK_INST.append((_g, _j, _bi))
MASK_IDX = {(g, j): (idx, bi) for idx, (g, j, bi) in enumerate(MASK_INST)}
assert len(MASK_INST) == 16


def _width(g, j):
    return 128 * sum(1 for L in GLS[g] if L > j)


_nc_cache = {}


def _build_nc():
    if "nc" in _nc_cache:
        return _nc_cache["nc"]
    nc = bass.Bass(num_devices=8)

    # inputs (per-core)
    xT_d = nc.dram_tensor("xT", [E, T], BF, kind="ExternalInput")
    xTq_d = nc.dram_tensor("xTq", [E, NQ * P], BF, kind="ExternalInput")
    WqT_d = nc.dram_tensor("WqT", [E, E], BF, kind="ExternalInput")
    WkT_d = nc.dram_tensor("WkT", [E, E], BF, kind="ExternalInput")
    WvT_d = nc.dram_tensor("WvT", [E, E], BF, kind="ExternalInput")
    WpT_d = nc.dram_tensor("WpT", [E, E], BF, kind="ExternalInput")
    bqT_d = nc.dram_tensor("bqT", [P, EC], F32, kind="ExternalInput")
    bkT_d = nc.dram_tensor("bkT", [P, EC], F32, kind="ExternalInput")
    bv_d = nc.dram_tensor("bv_bc", [P, E], BF, kind="ExternalInput")
    bp_d = nc.dram_tensor("bp_bc", [P, E], BF, kind="ExternalInput")
    gm_d = nc.dram_tensor("gamma_bc", [P, E], BF, kind="ExternalInput")
    bt_d = nc.dram_tensor("beta_bc", [P, E], BF, kind="ExternalInput")
    mall_d = nc.dram_tensor("mall", [P, 16, P], BF, kind="ExternalInput")
    y_d = nc.dram_tensor("y", [NQ, P, E], F32, kind="ExternalOutput")

    with tile.TileContext(nc) as tc:
        with ExitStack() as ctx:
            consts = ctx.enter_context(tc.tile_pool(name="consts", bufs=1))
            big = ctx.enter_context(tc.tile_pool(name="big", bufs=1))
            wpool = ctx.enter_context(tc.tile_pool(name="wpool", bufs=1))
            # xT is only needed during the QKV phase; its pool is closed
            # before the attention working set is allocated
            _xtp_cm = tc.tile_pool(name="xtp", bufs=1)
            xtp = _xtp_cm.__enter__()
            _psA_cm = tc.tile_pool(name="psA", bufs=1, space="PSUM")
            ps = _psA_cm.__enter__()

            def load_w(dram, name, interleave_with=None):
                # two half-tiles in a 3-slot rotation: the next projection's
                # first half streams in while the previous one's second half
                # is still being consumed. interleave_with: list of
                # (half_idx, chunk_idx) -> callable issuing a companion DMA
                halves = []
                for hf in range(2):
                    w = wpool.tile(
                        [P, EC, E // 2], BF, tag="wh", bufs=3, name=f"{name}{hf}"
                    )
                    for c in range(EC):
                        nc.sync.dma_start(
                            w[:, c, :],
                            dram.rearrange("(c p) f -> p c f", p=P)[
                                :, c, hf * 512 : (hf + 1) * 512
                            ],
                        )
                        if interleave_with is not None:
                            interleave_with(hf, c)
                    halves.append(w)
                return halves

            # PE-critical loads first: Wk half0 interleaved with xT window 0
            # (both needed by the first K matmul group), then Wk half1, then
            # the remaining xT windows — the ts-outer K loop consumes one
            # window per 13.7us so the DMA stream stays ahead
            xT = xtp.tile([P, EC, T], BF)

            def _xt_dma(hf, c):
                if hf == 0:
                    nc.sync.dma_start(
                        xT[:, c, 0:512],
                        xT_d.rearrange("(c p) t -> p c t", p=P)[:, c, 0:512],
                    )

            Wk = load_w(WkT_d, "Wk", interleave_with=_xt_dma)
            for wi in (1, 2, 3):
                for c in range(EC):
                    nc.sync.dma_start(
                        xT[:, c, wi * 512 : (wi + 1) * 512],
                        xT_d.rearrange("(c p) t -> p c t", p=P)[
                            :, c, wi * 512 : (wi + 1) * 512
                        ],
                    )
            bkT = consts.tile([P, EC], F32)
            nc.sync.dma_start(bkT[:], bkT_d[:, :])
            bv_bc = consts.tile([P, E], BF)
            nc.sync.dma_start(bv_bc[:], bv_d[:, :])
            xTq = big.tile([P, EC, NQ * P], BF)
            for c in range(EC):
                nc.sync.dma_start(
                    xTq[:, c, :], xTq_d.rearrange("(c p) t -> p c t", p=P)[:, c, :]
                )
            bqT = consts.tile([P, EC], F32)
            nc.sync.dma_start(bqT[:], bqT_d[:, :])
            bp_bc = consts.tile([P, E], BF)
            nc.sync.dma_start(bp_bc[:], bp_d[:, :])
            gamma_bc = consts.tile([P, E], BF)
            nc.sync.dma_start(gamma_bc[:], gm_d[:, :])
            beta_bc = consts.tile([P, E], BF)
            nc.sync.dma_start(beta_bc[:], bt_d[:, :])
            mall = consts.tile([P, 16, P], BF)
            nc.sync.dma_start(mall[:], mall_d[:, :, :])
            ones64 = consts.tile([P, 64], F32)
            nc.vector.memset(ones64[:], 1.0)

            # persistent intermediates
            KT = big.tile([P, EC, T], BF)  # K^T  [f, t]
            QT = big.tile([P, EC, NQ * P], BF)  # Q^T  [f, t_own]
            Vx = big.tile([P, NB, H, HD + 1], BF)  # V ext [t, h, d|1]
            zT = big.tile([P, EC, NQ * P], BF)  # z^T  [e, t_own]
            nc.vector.memset(Vx[:, :, :, HD : HD + 1], 1.0)

            # ---------------- K^T = Wk^T.T-chunks x xT + bk ----------------
            # ts outer: each xT window feeds all 8 fb groups (~13.7us of PE
            # work) so the next window's DMA completes in the shadow
            for ts_ in range(T // 512):
                for fb in range(EC):
                    pk = ps.tile([P, 512], F32, tag="mm512", bufs=4, name="pk")
                    for c in range(EC):
                        nc.tensor.matmul(
                            pk[:],
                            Wk[fb // 4][:, c, (fb % 4) * P : (fb % 4 + 1) * P],
                            xT[:, c, ts_ * 512 : (ts_ + 1) * 512],
                            start=(c == 0),
                            stop=(c == EC - 1),
                        )
                    nc.vector.tensor_scalar(
                        out=KT[:, fb, ts_ * 512 : (ts_ + 1) * 512],
                        in0=pk[:],
                        scalar1=bkT[:, fb : fb + 1],
                        scalar2=None,
                        op0=mybir.AluOpType.add,
                    )

            # ---------------- V = xT-chunks x Wv^T + bv (t-major, ext) -----
            Wv = load_w(WvT_d, "Wv")
            for tb in range(NB):
                for fs in range(E // 512):
                    pv = ps.tile([P, 512], F32, tag="mm512", bufs=4, name="pv")
                    for c in range(EC):
                        nc.tensor.matmul(
                            pv[:],
                            xT[:, c, tb * P : (tb + 1) * P],
                            Wv[fs][:, c, :],
                            start=(c == 0),
                            stop=(c == EC - 1),
                        )
                    nc.vector.tensor_tensor(
                        out=Vx[:, tb, fs * 8 : (fs + 1) * 8, 0:HD],
                        in0=pv[:, :].rearrange("p (h d) -> p h d", d=HD),
                        in1=bv_bc[:, fs * 512 : (fs + 1) * 512].rearrange(
                            "p (h d) -> p h d", d=HD
                        ),
                        op=mybir.AluOpType.add,
                    )

            # ---------------- Q^T = Wq^T-chunks x xTq + bq -----------------
            Wq = load_w(WqT_d, "Wq")
            for fb in range(EC):
                for ts_ in range(NQ * P // 512):
                    pq = ps.tile([P, 512], F32, tag="mm512", bufs=4, name="pq")
                    for c in range(EC):
                        nc.tensor.matmul(
                            pq[:],
                            Wq[fb // 4][:, c, (fb % 4) * P : (fb % 4 + 1) * P],
                            xTq[:, c, ts_ * 512 : (ts_ + 1) * 512],
                            start=(c == 0),
                            stop=(c == EC - 1),
                        )
                    nc.vector.tensor_scalar(
                        out=QT[:, fb, ts_ * 512 : (ts_ + 1) * 512],
                        in0=pq[:],
                        scalar1=bqT[:, fb : fb + 1],
                        scalar2=None,
                        op0=mybir.AluOpType.add,
                    )

            # Wp streams in during attention (3-slot rotation frees Wq slots)
            Wp = load_w(WpT_d, "Wp")

            # ---------------- attention ----------------
            # xT dead from here; free its SBUF for the attention working set
            _xtp_cm.__exit__(None, None, None)
            work = ctx.enter_context(tc.tile_pool(name="work", bufs=2))
            _psA_cm.__exit__(None, None, None)
            _psB_cm = tc.tile_pool(name="psB", bufs=1, space="PSUM")
            ps = _psB_cm.__enter__()

            # units: (group, head, pair-of-key-blocks); group 0 (heavy) first,
            # then group 1 with the first 4 projection token-blocks interleaved
            units = []
            for g in (0, 1):
                for h in range(H):
                    for p_ in range(GLS[g][0] // 2):
                        units.append((g, h, p_))
            FIRST_G1 = 16 * 8

            def emit_S(u):
                g, h, p_ = units[u]
                j0 = 2 * p_
                w = _width(g, j0)
                hb = (h % 2) * 64
                pS = ps.tile([P, 1024], F32, tag="pS", bufs=2, name="pS")
                for jj in (0, 1):
                    j = j0 + jj
                    nc.tensor.matmul(
                        pS[:, jj * 512 : jj * 512 + w],
                        KT[hb : hb + 64, h // 2, j * P : (j + 1) * P],
                        QT[hb : hb + 64, h // 2, g * 512 : g * 512 + w],
                        start=True,
                        stop=True,
                    )
                return pS

            def emit_division(h, g, pO):
                hb = (h % 2) * 64
                # reciprocal of the denominators row (accumulated via the
                # Vx ones column), broadcast across 64 partitions with a
                # K=1 matmul into the bank's unused upper rows, one multiply
                rr = work.tile([P, 512], F32, tag="rr", bufs=2, name="rr")
                nc.vector.reciprocal(rr[64:65, :], pO[64:65, :])
                nc.tensor.matmul(
                    pO[64:128, :], ones64[64:65, :], rr[64:65, :],
                    start=True, stop=True,
                )
                zh = work.tile([64, 512], BF, tag="zh", bufs=3, name="zh")
                nc.vector.tensor_tensor(
                    out=zh[:], in0=pO[0:HD, :], in1=pO[64:128, :],
                    op=mybir.AluOpType.mult,
                )
                nc.sync.dma_start(
                    zT[hb : hb + 64, h // 2, g * 512 : (g + 1) * 512], zh[:]
                )

            def emit_residual(g):
                cols = slice(g * 512, (g + 1) * 512)
                for c in range(EC):
                    nc.vector.tensor_tensor(
                        out=zT[:, c, cols], in0=zT[:, c, cols],
                        in1=xTq[:, c, cols], op=mybir.AluOpType.add,
                    )

            inv_e = 1.0 / float(E)

            def emit_proj_tb(tb, last=False):
                # bias-add fused with row-sum accumulation (mean), variance
                # via Square(y - mu) with accum, final normalize as one
                # scale+bias activation; gamma/beta on the idle Pool engine
                # except for the last block (shortest critical chain on DVE)
                y_sb = work.tile([P, E], F32, tag="ysb", bufs=2, name="y_sb")
                s0 = work.tile([P, 1], F32, tag="stat", bufs=16, name="s0")
                ysum = work.tile([P, 1], F32, tag="stat", bufs=16, name="ysum")
                for fs in range(E // 512):
                    py = ps.tile([P, 512], F32, tag="py", bufs=2, name="py")
                    for c in range(EC):
                        nc.tensor.matmul(
                            py[:],
                            zT[:, c, tb * P : (tb + 1) * P],
                            Wp[fs][:, c, :],
                            start=(c == 0),
                            stop=(c == EC - 1),
                        )
                    nc.vector.tensor_tensor_reduce(
                        out=y_sb[:, fs * 512 : (fs + 1) * 512],
                        in0=py[:],
                        in1=bp_bc[:, fs * 512 : (fs + 1) * 512],
                        scale=1.0,
                        scalar=(0.0 if fs == 0 else s0[:, 0:1]),
                        op0=mybir.AluOpType.add,
                        op1=mybir.AluOpType.add,
                        accum_out=(s0 if fs == 0 else ysum)[:, 0:1],
                    )
                negmu = work.tile([P, 1], F32, tag="stat", bufs=16, name="negmu")
                nc.vector.tensor_scalar_mul(negmu[:], ysum[:], -inv_e)
                y_c = work.tile([P, E], F32, tag="yc", bufs=2, name="y_c")
                var = work.tile([P, 1], F32, tag="stat", bufs=16, name="var")
                nc.scalar.activation(
                    y_c[:], y_sb[:], mybir.ActivationFunctionType.Square,
                    bias=negmu[:, 0:1], accum_out=var[:],
                )
                rstd = work.tile([P, 1], F32, tag="stat", bufs=16, name="rstd")
                nc.vector.tensor_scalar(
                    out=rstd[:], in0=var[:], scalar1=inv_e, scalar2=float(EPS),
                    op0=mybir.AluOpType.mult, op1=mybir.AluOpType.add,
                )
                nc.scalar.activation(
                    rstd[:], rstd[:], mybir.ActivationFunctionType.Sqrt
                )
                nc.vector.reciprocal(rstd[:], rstd[:])
                nmr = work.tile([P, 1], F32, tag="stat", bufs=16, name="nmr")
                nc.vector.tensor_tensor(
                    out=nmr[:], in0=negmu[:], in1=rstd[:], op=mybir.AluOpType.mult
                )
                nc.scalar.activation(
                    y_c[:], y_sb[:], mybir.ActivationFunctionType.Identity,
                    scale=rstd[:, 0:1], bias=nmr[:, 0:1],
                )
                eng = nc.vector if last else nc.gpsimd
                eng.tensor_tensor(
                    out=y_sb[:], in0=y_c[:], in1=gamma_bc[:],
                    op=mybir.AluOpType.mult,
                )
                eng.tensor_tensor(
                    out=y_sb[:], in0=y_sb[:], in1=beta_bc[:],
                    op=mybir.AluOpType.add,
                )
                nc.sync.dma_start(y_d[tb, :, :], y_sb[:])

            # interleave projection token-blocks 0..3 into the light group 1
            PROJ_AT = {12: 0, 24: 1, 36: 2, 48: 3}

            pO_cur = None
            pending_div = None
            prev_S = emit_S(0)
            for u, (g, h, p_) in enumerate(units):
                j0 = 2 * p_
                w = _width(g, j0)
                maxL = GLS[g][0]
                if p_ == 0:
                    pO_cur = ps.tile([P, 512], F32, tag="pO", bufs=2, name="pO")
                pO = pO_cur
                pS = prev_S
                eS = work.tile([P, 1024], BF, tag="eS", bufs=3, name="eS")
                nc.scalar.activation(
                    eS[:, :].rearrange("p (u q) -> p u q", u=2)[:, :, 0:w],
                    pS[:, :].rearrange("p (u q) -> p u q", u=2)[:, :, 0:w],
                    mybir.ActivationFunctionType.Exp,
                    scale=SCALE,
                )
                if u + 1 < len(units):
                    prev_S = emit_S(u + 1)
                if pending_div is not None and p_ == 0:
                    pending_div()
                    pending_div = None
                if u == FIRST_G1:
                    emit_residual(0)
                if u - FIRST_G1 in PROJ_AT:
                    emit_proj_tb(PROJ_AT[u - FIRST_G1])
                for jj in (0, 1):
                    j = j0 + jj
                    mi = MASK_IDX.get((g, j))
                    if mi is not None:
                        idx, bi = mi
                        cs = slice(jj * 512 + bi * P, jj * 512 + (bi + 1) * P)
                        nc.vector.tensor_tensor(
                            out=eS[:, cs], in0=eS[:, cs],
                            in1=mall[:, idx, :], op=mybir.AluOpType.mult,
                        )
                    nc.tensor.matmul(
                        pO[0 : HD + 1, 0:w],
                        Vx[:, j, h, :],
                        eS[:, jj * 512 : jj * 512 + w],
                        start=(j == 0),
                        stop=(j == maxL - 1),
                        skip_group_check=True,
                    )
                if j0 + 1 == maxL - 1:

                    def _div(h=h, g=g, pO=pO):
                        emit_division(h, g, pO)

                    pending_div = _div
            if pending_div is not None:
                pending_div()
                pending_div = None

            # ---------------- tail: residual + projection for group 1 ------
            emit_residual(1)
            for tb in range(4, NQ):
                emit_proj_tb(tb, last=(tb == NQ - 1))

            _psB_cm.__exit__(None, None, None)

    _nc_cache["nc"] = nc
    return nc


def _make_mall(ownd):
    """Mask tiles for this core's descending-ordered q-blocks.

    Instance (g, j, bi): multiply eS columns of boundary q-block bi at key
    block j. Pattern depends on whether the block's true length equals the
    padded length (l_true == L) or falls one short (l_true == L-1)."""
    tril_t = (np.arange(P)[:, None] <= np.arange(P)[None, :]).astype(np.float32)
    mall = np.zeros((16, P, P), np.float32)
    for idx, (g, j, bi) in enumerate(MASK_INST):
        L = GLS[g][bi]
        block = ownd[g * 4 + bi]
        l_true = block + 1
        assert l_true in (L, L - 1)
        if j == L - 2:
            mall[idx] = 1.0 if l_true == L else tril_t
        else:
            mall[idx] = tril_t if l_true == L else 0.0
    # device layout [P(k-local), 16, P(q-local)]
    return np.ascontiguousarray(mall.transpose(1, 0, 2)).astype(NPBF)


def kernel(x, Wq, bq, Wk, bk, Wv, bv, Wp, bp, gamma, beta):
    x = np.asarray(x, np.float32)
    nc = _build_nc()

    WqT = np.ascontiguousarray(np.asarray(Wq, np.float32).T).astype(NPBF)
    WkT = np.ascontiguousarray(np.asarray(Wk, np.float32).T).astype(NPBF)
    WvT = np.ascontiguousarray(np.asarray(Wv, np.float32).T).astype(NPBF)
    WpT = np.ascontiguousarray(np.asarray(Wp, np.float32).T).astype(NPBF)
    bqT = np.ascontiguousarray(np.asarray(bq, np.float32).reshape(EC, P).T)
    bkT = np.ascontiguousarray(np.asarray(bk, np.float32).reshape(EC, P).T)
    bv_bc = np.ascontiguousarray(
        np.broadcast_to(np.asarray(bv, np.float32), (P, E))
    ).astype(NPBF)
    bp_bc = np.ascontiguousarray(
        np.broadcast_to(np.asarray(bp, np.float32), (P, E))
    ).astype(NPBF)
    gamma_bc = np.ascontiguousarray(
        np.broadcast_to(np.asarray(gamma, np.float32), (P, E))
    ).astype(NPBF)
    beta_bc = np.ascontiguousarray(
        np.broadcast_to(np.asarray(beta, np.float32), (P, E))
    ).astype(NPBF)
    # descending padded length = reversed block list
    ownd_map = {0: list(reversed(BLOCKS_A)), 1: list(reversed(BLOCKS_B))}
    mall_map = {hh: _make_mall(ownd_map[hh]) for hh in (0, 1)}

    in_maps = []
    for core in range(8):
        b, hh = core // 2, core % 2
        ownd = ownd_map[hh]
        own = np.concatenate([np.arange(blk * P, (blk + 1) * P) for blk in ownd])
        xb = x[b]  # (T, E)
        xT = np.ascontiguousarray(xb.T).astype(NPBF)
        xTq = np.ascontiguousarray(xb[own].T).astype(NPBF)
        in_maps.append(
            {
                "xT": xT,
                "xTq": xTq,
                "WqT": WqT,
                "WkT": WkT,
                "WvT": WvT,
                "WpT": WpT,
                "bqT": bqT,
                "bkT": bkT,
                "bv_bc": bv_bc,
                "bp_bc": bp_bc,
                "gamma_bc": gamma_bc,
                "beta_bc": beta_bc,
                "mall": mall_map[hh],
            }
        )

    import os

    trace = bool(int(os.environ.get("MHSA_TRACE", "0")))
    res = run_bass_kernel_spmd(
        nc, in_maps, core_ids=list(range(8)), trace=trace,
        trace_cores=list(range(8)) if trace else None,
    )
    if trace and res.exec_time_ns is not None:
        print(f"HW exec time: {res.exec_time_ns} ns")
        if res.mean_exec_time_ns is not None:
            print(f"HW exec mean across cores: {res.mean_exec_time_ns:.0f} ns")
        kernel.last_exec_time_ns = res.exec_time_ns
        kernel.last_trace = res.instructions_and_trace

    out = np.empty((B, T, E), np.float32)
    for core in range(8):
        b, hh = core // 2, core % 2
        ownd = ownd_map[hh]
        y = res.results[core]["y"]  # (NQ, P, E)
        for k, blk in enumerate(ownd):
            out[b, blk * P : (blk + 1) * P, :] = y[k]
    return out


# revision 23
# speedup vs baseline: 1.0554x; 1.0554x over previous
"""Multi-head self-attention (B=4, T=2048, E=1024, H=16) on 8 trn2 NeuronCores.

Sharding: core (b, h) = batch b, token-half h. Each core computes K/V for the
full sequence (duplicated within the batch pair), Q for its own 8 query blocks
of 128 tokens, causal attention for those blocks, then the output projection
and LayerNorm for its own tokens.

Attention restructure (vs the 128-wide-per-head-pair baseline): each core's
query blocks are ordered by DESCENDING padded causal length (16,14,12,10 |
8,6,4,2 key blocks), so for key block j the active query blocks form a
contiguous prefix. Scores/AV run one matmul per (head, group-of-4-q-blocks,
key block) with free dim up to 512, cutting PE instruction count ~3x. The
softmax denominator division runs once per (head, group) on 512 columns.
Projection+LN for the first 4 token blocks is interleaved into the second
(light) attention group to shrink the tail.

Causal balance: query blocks are paired (j, 15-j) so both cores of a batch
process blocks with padded key-lengths 2,4,...,16; host-supplied mask tiles
encode the true causal structure, keeping the compiled program identical
across cores (SPMD).

All matmuls run in bf16 with fp32 PSUM accumulation (validated ~2e-3
scale-relative error vs the fp32 reference).
"""
import json
import numpy as np
import ml_dtypes
from contextlib import ExitStack

import concourse.bass as bass
import concourse.bass_utils as _bass_utils
import concourse.tile as tile
from concourse import mybir
from concourse.bass_utils import run_bass_kernel_spmd

# ----------------------------------------------------------------------------
# Toolchain workarounds for this container's walrus build (see birfix notes):
# 1. EVENT_SEMAPHORE_RANGE_CLEAR InstISA is rejected ("ISA wrong length").
# 2. Engine instructions only carry one semaphore-wait slot; extra waits are
#    peeled onto NoOp carriers on the same engine (order-preserving).
# ----------------------------------------------------------------------------


def _patched_clear_and_free_semaphores(self, sems):
    if not sems:
        return
    sem_nums = [s.num if hasattr(s, "num") else s for s in sems]
    self._state.prepend_free_semaphores(sem_nums)
    for poison_set in self._tile_sem_poison_stack:
        poison_set.update(sem_nums)


def _fix_bir_waits(bir_json: bytes) -> bytes:
    bir = json.loads(bir_json)
    ctr = 0
    changed = False
    for func in bir.get("functions", []):
        for blk in func.get("blocks", []):
            out = []
            for inst in blk.get("instructions", []):
                si = inst.get("sync_info") or {}
                waits = si.get("on_wait") or []
                if len(waits) > 1:
                    for w in waits[:-1]:
                        ctr += 1
                        out.append(
                            {
                                "debug": inst.get("debug"),
                                "engine": inst.get("engine", "SP"),
                                "ins": [],
                                "name": f"IWF-{ctr}",
                                "opcode": "NoOp",
                                "outs": [],
                                "sync_info": {"on_wait": [w]},
                            }
                        )
                    si = dict(si)
                    si["on_wait"] = waits[-1:]
                    inst = dict(inst)
                    inst["sync_info"] = si
                    changed = True
                out.append(inst)
            blk["instructions"] = out
    return json.dumps(bir).encode() if changed else bir_json


_orig_compile_bir_kernel = _bass_utils.compile_bir_kernel


def _patched_compile_bir_kernel(bir_json, tmpdir, neff_name="file.neff"):
    if isinstance(bir_json, str):
        bir_json = bir_json.encode()
    return _orig_compile_bir_kernel(_fix_bir_waits(bir_json), tmpdir, neff_name)


def _install_patches():
    if getattr(bass.Bass, "_mhsa_patched", False):
        return
    bass.Bass.clear_and_free_semaphores = _patched_clear_and_free_semaphores
    bass.Bass._mhsa_patched = True
    _bass_utils.compile_bir_kernel = _patched_compile_bir_kernel
    try:
        import concourse.bass2jax as _b2j

        _b2j.compile_bir_kernel = _patched_compile_bir_kernel
    except ImportError:
        pass


_install_patches()

# ----------------------------------------------------------------------------
# Problem constants (hardcoded per spec)
# ----------------------------------------------------------------------------
B, T, E, H = 4, 2048, 1024, 16
HD = E // H  # 64
P = 128
NB = T // P  # 16 query/key blocks
NQ = 8  # query blocks per core
EC = E // P  # 8 e-chunks
SCALE = 1.0 / float(np.sqrt(T))
EPS = 1e-6
BF = mybir.dt.bfloat16
F32 = mybir.dt.float32
NPBF = ml_dtypes.bfloat16

# query-block assignment: pairs (j, 15-j) so both cores of a batch pair see
# padded lengths {2,4,...,16}; blocks listed in ASCENDING padded length
BLOCKS_A = [0, 2, 4, 6, 9, 11, 13, 15]  # true lengths 1,3,5,7,10,12,14,16
BLOCKS_B = [1, 3, 5, 7, 8, 10, 12, 14]  # true lengths 2,4,6,8,9,11,13,15

# device-side q-block order: DESCENDING padded length; two groups of 4
GLS = {0: (16, 14, 12, 10), 1: (8, 6, 4, 2)}
# mask instances: (group, key block j, boundary q-block index bi); the
# boundary block is always the LAST active block of the prefix at that j
MASK_INST = []
for _g in (0, 1):
    for _j in range(GLS[_g][0]):
        for _bi, _L in enumerate(GLS[_g]):
            if _j in (_L - 2, _L - 1):
                MASK_INST.append((_g, _j, _bi))
MASK_IDX = {(g, j): (idx, bi) for idx, (g, j, bi) in enumerate(MASK_INST)}
assert len(MASK_INST) == 16


def _width(g, j):
    return 128 * sum(1 for L in GLS[g] if L > j)


_nc_cache = {}


def _build_nc():
    if "nc" in _nc_cache:
        return _nc_cache["nc"]
    nc = bass.Bass(num_devices=8)

    # inputs (per-core)
    xT_d = nc.dram_tensor("xT", [E, T], BF, kind="ExternalInput")
    xTq_d = nc.dram_tensor("xTq", [E, NQ * P], BF, kind="ExternalInput")
    WqT_d = nc.dram_tensor("WqT", [E, E], BF, kind="ExternalInput")
    WkT_d = nc.dram_tensor("WkT", [E, E], BF, kind="ExternalInput")
    WvT_d = nc.dram_tensor("WvT", [E, E], BF, kind="ExternalInput")
    WpT_d = nc.dram_tensor("WpT", [E, E], BF, kind="ExternalInput")
    cF32_d = nc.dram_tensor("cF32", [P, 16], F32, kind="ExternalInput")
    cBF_d = nc.dram_tensor("cBF", [P, 4 * E + 16 * P], BF, kind="ExternalInput")
    y_d = nc.dram_tensor("y", [NQ, P, E], F32, kind="ExternalOutput")

    with tile.TileContext(nc) as tc:
        with ExitStack() as ctx:
            consts = ctx.enter_context(tc.tile_pool(name="consts", bufs=1))
            big = ctx.enter_context(tc.tile_pool(name="big", bufs=1))
            wpool = ctx.enter_context(tc.tile_pool(name="wpool", bufs=1))
            # xT is only needed during the QKV phase; its pool is closed
            # before the attention working set is allocated
            _xtp_cm = tc.tile_pool(name="xtp", bufs=1)
            xtp = _xtp_cm.__enter__()
            _psA_cm = tc.tile_pool(name="psA", bufs=1, space="PSUM")
            ps = _psA_cm.__enter__()

            def load_w(dram, name, interleave_with=None):
                # two half-tiles in a 3-slot rotation: the next projection's
                # first half streams in while the previous one's second half
                # is still being consumed. ONE DMA per half (HWDGE issue is a
                # serialized ~625ns/DMA shared resource — minimize count)
                halves = []
                for hf in range(2):
                    w = wpool.tile(
                        [P, EC, E // 2], BF, tag="wh", bufs=3, name=f"{name}{hf}"
                    )
                    nc.sync.dma_start(
                        w[:, :, :],
                        dram.rearrange("(c p) f -> p c f", p=P)[
                            :, :, hf * 512 : (hf + 1) * 512
                        ],
                    )
                    if interleave_with is not None:
                        interleave_with(hf)
                    halves.append(w)
                return halves

            # PE-critical loads first. HWDGE queue order: Wk half0, xT win0,
            # f32 consts (bk needed by the first bias add), Wk half1, then
            # the remaining xT windows — the ts-outer K loop consumes one
            # window per ~13.7us so the serialized DMA stream stays ahead
            xT = xtp.tile([P, EC, T], BF)
            cF32 = consts.tile([P, 16], F32)
            cBF = consts.tile([P, 4 * E + 16 * P], BF)

            def _xt_w(wi):
                nc.sync.dma_start(
                    xT[:, :, wi * 512 : (wi + 1) * 512],
                    xT_d.rearrange("(c p) t -> p c t", p=P)[
                        :, :, wi * 512 : (wi + 1) * 512
                    ],
                )

            def _wk_companion(hf):
                if hf == 0:
                    _xt_w(0)
                    nc.sync.dma_start(cF32[:, :], cF32_d[:, :])

            Wk = load_w(WkT_d, "Wk", interleave_with=_wk_companion)
            for wi in (1, 2, 3):
                _xt_w(wi)
            xTq = big.tile([P, EC, NQ * P], BF)
            nc.sync.dma_start(
                xTq[:, :, :], xTq_d.rearrange("(c p) t -> p c t", p=P)[:, :, :]
            )
            nc.sync.dma_start(cBF[:, :], cBF_d[:, :])
            # packed-constant layout in cBF: bv | bp | gamma | beta | masks
            OFF_BV, OFF_BP, OFF_G, OFF_B, OFF_M = 0, E, 2 * E, 3 * E, 4 * E

            def mall_at(idx):
                return cBF[:, OFF_M + idx * P : OFF_M + (idx + 1) * P]

            ones64 = consts.tile([P, 64], F32)
            nc.vector.memset(ones64[:], 1.0)

            # persistent intermediates
            KT = big.tile([P, EC, T], BF)  # K^T  [f, t]
            QT = big.tile([P, EC, NQ * P], BF)  # Q^T  [f, t_own]
            Vx = big.tile([P, NB, H, HD + 1], BF)  # V ext [t, h, d|1]
            zT = big.tile([P, EC, NQ * P], BF)  # z^T  [e, t_own]
            nc.vector.memset(Vx[:, :, :, HD : HD + 1], 1.0)

            # ---------------- K^T = Wk^T.T-chunks x xT + bk ----------------
            # ts outer: each xT window feeds all 8 fb groups (~13.7us of PE
            # work) so the next window's DMA completes in the shadow
            for ts_ in range(T // 512):
                for fb in range(EC):
                    pk = ps.tile([P, 512], F32, tag="mm512", bufs=4, name="pk")
                    for c in range(EC):
                        nc.tensor.matmul(
                            pk[:],
                            Wk[fb // 4][:, c, (fb % 4) * P : (fb % 4 + 1) * P],
                            xT[:, c, ts_ * 512 : (ts_ + 1) * 512],
                            start=(c == 0),
                            stop=(c == EC - 1),
                        )
                    nc.vector.tensor_scalar(
                        out=KT[:, fb, ts_ * 512 : (ts_ + 1) * 512],
                        in0=pk[:],
                        scalar1=cF32[:, fb : fb + 1],
                        scalar2=None,
                        op0=mybir.AluOpType.add,
                    )

            # ---------------- V = xT-chunks x Wv^T + bv (t-major, ext) -----
            Wv = load_w(WvT_d, "Wv")
            for tb in range(NB):
                for fs in range(E // 512):
                    pv = ps.tile([P, 512], F32, tag="mm512", bufs=4, name="pv")
                    for c in range(EC):
                        nc.tensor.matmul(
                            pv[:],
                            xT[:, c, tb * P : (tb + 1) * P],
                            Wv[fs][:, c, :],
                            start=(c == 0),
                            stop=(c == EC - 1),
                        )
                    nc.vector.tensor_tensor(
                        out=Vx[:, tb, fs * 8 : (fs + 1) * 8, 0:HD],
                        in0=pv[:, :].rearrange("p (h d) -> p h d", d=HD),
                        in1=cBF[:, OFF_BV + fs * 512 : OFF_BV + (fs + 1) * 512].rearrange(
                            "p (h d) -> p h d", d=HD
                        ),
                        op=mybir.AluOpType.add,
                    )

            # ---------------- Q^T = Wq^T-chunks x xTq + bq -----------------
            Wq = load_w(WqT_d, "Wq")
            for fb in range(EC):
                for ts_ in range(NQ * P // 512):
                    pq = ps.tile([P, 512], F32, tag="mm512", bufs=4, name="pq")
                    for c in range(EC):
                        nc.tensor.matmul(
                            pq[:],
                            Wq[fb // 4][:, c, (fb % 4) * P : (fb % 4 + 1) * P],
                            xTq[:, c, ts_ * 512 : (ts_ + 1) * 512],
                            start=(c == 0),
                            stop=(c == EC - 1),
                        )
                    nc.vector.tensor_scalar(
                        out=QT[:, fb, ts_ * 512 : (ts_ + 1) * 512],
                        in0=pq[:],
                        scalar1=cF32[:, EC + fb : EC + fb + 1],
                        scalar2=None,
                        op0=mybir.AluOpType.add,
                    )

            # Wp streams in during attention (3-slot rotation frees Wq slots)
            Wp = load_w(WpT_d, "Wp")

            # ---------------- attention ----------------
            # xT dead from here; free its SBUF for the attention working set
            _xtp_cm.__exit__(None, None, None)
            work = ctx.enter_context(tc.tile_pool(name="work", bufs=2))
            _psA_cm.__exit__(None, None, None)
            _psB_cm = tc.tile_pool(name="psB", bufs=1, space="PSUM")
            ps = _psB_cm.__enter__()

            # units: (group, head, pair-of-key-blocks); group 0 (heavy) first,
            # then group 1 with the first 4 projection token-blocks interleaved
            units = []
            for g in (0, 1):
                for h in range(H):
                    for p_ in range(GLS[g][0] // 2):
                        units.append((g, h, p_))
            FIRST_G1 = 16 * 8

            def emit_S(u):
                g, h, p_ = units[u]
                j0 = 2 * p_
                w = _width(g, j0)
                hb = (h % 2) * 64
                pS = ps.tile([P, 1024], F32, tag="pS", bufs=2, name="pS")
                for jj in (0, 1):
                    j = j0 + jj
                    nc.tensor.matmul(
                        pS[:, jj * 512 : jj * 512 + w],
                        KT[hb : hb + 64, h // 2, j * P : (j + 1) * P],
                        QT[hb : hb + 64, h // 2, g * 512 : g * 512 + w],
                        start=True,
                        stop=True,
                    )
                return pS

            def emit_division(h, g, pO):
                hb = (h % 2) * 64
                # reciprocal of the denominators row (accumulated via the
                # Vx ones column), broadcast across 64 partitions with a
                # K=1 matmul into the bank's unused upper rows, one multiply
                rr = work.tile([P, 512], F32, tag="rr", bufs=2, name="rr")
                nc.vector.reciprocal(rr[64:65, :], pO[64:65, :])
                nc.tensor.matmul(
                    pO[64:128, :], ones64[64:65, :], rr[64:65, :],
                    start=True, stop=True,
                )
                # write the normalized head output straight into z^T (saves
                # a serialized HWDGE slot per (head, group))
                nc.vector.tensor_tensor(
                    out=zT[hb : hb + 64, h // 2, g * 512 : (g + 1) * 512],
                    in0=pO[0:HD, :], in1=pO[64:128, :],
                    op=mybir.AluOpType.mult,
                )

            def emit_residual(g):
                cols = slice(g * 512, (g + 1) * 512)
                for c in range(EC):
                    nc.vector.tensor_tensor(
                        out=zT[:, c, cols], in0=zT[:, c, cols],
                        in1=xTq[:, c, cols], op=mybir.AluOpType.add,
                    )

            inv_e = 1.0 / float(E)

            def emit_proj_tb(tb, last=False):
                # bias-add fused with row-sum accumulation (mean), variance
                # via Square(y - mu) with accum, final normalize as one
                # scale+bias activation; gamma/beta on the idle Pool engine
                # except for the last block (shortest critical chain on DVE)
                y_sb = work.tile([P, E], F32, tag="ysb", bufs=2, name="y_sb")
                s0 = work.tile([P, 1], F32, tag="stat", bufs=16, name="s0")
                ysum = work.tile([P, 1], F32, tag="stat", bufs=16, name="ysum")
                for fs in range(E // 512):
                    py = ps.tile([P, 512], F32, tag="py", bufs=2, name="py")
                    for c in range(EC):
                        nc.tensor.matmul(
                            py[:],
                            zT[:, c, tb * P : (tb + 1) * P],
                            Wp[fs][:, c, :],
                            start=(c == 0),
                            stop=(c == EC - 1),
                        )
                    nc.vector.tensor_tensor_reduce(
                        out=y_sb[:, fs * 512 : (fs + 1) * 512],
                        in0=py[:],
                        in1=cBF[:, OFF_BP + fs * 512 : OFF_BP + (fs + 1) * 512],
                        scale=1.0,
                        scalar=(0.0 if fs == 0 else s0[:, 0:1]),
                        op0=mybir.AluOpType.add,
                        op1=mybir.AluOpType.add,
                        accum_out=(s0 if fs == 0 else ysum)[:, 0:1],
                    )
                negmu = work.tile([P, 1], F32, tag="stat", bufs=16, name="negmu")
                nc.vector.tensor_scalar_mul(negmu[:], ysum[:], -inv_e)
                y_c = work.tile([P, E], F32, tag="yc", bufs=2, name="y_c")
                var = work.tile([P, 1], F32, tag="stat", bufs=16, name="var")
                nc.scalar.activation(
                    y_c[:], y_sb[:], mybir.ActivationFunctionType.Square,
                    bias=negmu[:, 0:1], accum_out=var[:],
                )
                rstd = work.tile([P, 1], F32, tag="stat", bufs=16, name="rstd")
                nc.vector.tensor_scalar(
                    out=rstd[:], in0=var[:], scalar1=inv_e, scalar2=float(EPS),
                    op0=mybir.AluOpType.mult, op1=mybir.AluOpType.add,
                )
                nc.scalar.activation(
                    rstd[:], rstd[:], mybir.ActivationFunctionType.Sqrt
                )
                nc.vector.reciprocal(rstd[:], rstd[:])
                nmr = work.tile([P, 1], F32, tag="stat", bufs=16, name="nmr")
                nc.vector.tensor_tensor(
                    out=nmr[:], in0=negmu[:], in1=rstd[:], op=mybir.AluOpType.mult
                )
                nc.scalar.activation(
                    y_c[:], y_sb[:], mybir.ActivationFunctionType.Identity,
                    scale=rstd[:, 0:1], bias=nmr[:, 0:1],
                )
                eng = nc.vector if last else nc.gpsimd
                eng.tensor_tensor(
                    out=y_sb[:], in0=y_c[:], in1=cBF[:, OFF_G : OFF_G + E],
                    op=mybir.AluOpType.mult,
                )
                eng.tensor_tensor(
                    out=y_sb[:], in0=y_sb[:], in1=cBF[:, OFF_B : OFF_B + E],
                    op=mybir.AluOpType.add,
                )
                nc.sync.dma_start(y_d[tb, :, :], y_sb[:])

            # interleave projection token-blocks 0..3 into the light group 1
            PROJ_AT = {12: 0, 24: 1, 36: 2, 48: 3}

            pO_cur = None
            pending_div = None
            prev_S = emit_S(0)
            for u, (g, h, p_) in enumerate(units):
                j0 = 2 * p_
                w = _width(g, j0)
                maxL = GLS[g][0]
                if p_ == 0:
                    pO_cur = ps.tile([P, 512], F32, tag="pO", bufs=2, name="pO")
                pO = pO_cur
                pS = prev_S
                eS = work.tile([P, 1024], BF, tag="eS", bufs=3, name="eS")
                nc.scalar.activation(
                    eS[:, :].rearrange("p (u q) -> p u q", u=2)[:, :, 0:w],
                    pS[:, :].rearrange("p (u q) -> p u q", u=2)[:, :, 0:w],
                    mybir.ActivationFunctionType.Exp,
                    scale=SCALE,
                )
                if u + 1 < len(units):
                    prev_S = emit_S(u + 1)
                if pending_div is not None and p_ == 0:
                    pending_div()
                    pending_div = None
                if u == FIRST_G1:
                    emit_residual(0)
                if u - FIRST_G1 in PROJ_AT:
                    emit_proj_tb(PROJ_AT[u - FIRST_G1])
                for jj in (0, 1):
                    j = j0 + jj
                    mi = MASK_IDX.get((g, j))
                    if mi is not None:
                        idx, bi = mi
                        cs = slice(jj * 512 + bi * P, jj * 512 + (bi + 1) * P)
                        nc.vector.tensor_tensor(
                            out=eS[:, cs], in0=eS[:, cs],
                            in1=mall_at(idx), op=mybir.AluOpType.mult,
                        )
                    nc.tensor.matmul(
                        pO[0 : HD + 1, 0:w],
                        Vx[:, j, h, :],
                        eS[:, jj * 512 : jj * 512 + w],
                        start=(j == 0),
                        stop=(j == maxL - 1),
                        skip_group_check=True,
                    )
                if j0 + 1 == maxL - 1:

                    def _div(h=h, g=g, pO=pO):
                        emit_division(h, g, pO)

                    pending_div = _div
            if pending_div is not None:
                pending_div()
                pending_div = None

            # ---------------- tail: residual + projection for group 1 ------
            emit_residual(1)
            for tb in range(4, NQ):
                emit_proj_tb(tb, last=(tb == NQ - 1))

            _psB_cm.__exit__(None, None, None)

    _nc_cache["nc"] = nc
    return nc


def _make_mall(ownd):
    """Mask tiles for this core's descending-ordered q-blocks.

    Instance (g, j, bi): multiply eS columns of boundary q-block bi at key
    block j. Pattern depends on whether the block's true length equals the
    padded length (l_true == L) or falls one short (l_true == L-1)."""
    tril_t = (np.arange(P)[:, None] <= np.arange(P)[None, :]).astype(np.float32)
    mall = np.zeros((16, P, P), np.float32)
    for idx, (g, j, bi) in enumerate(MASK_INST):
        L = GLS[g][bi]
        block = ownd[g * 4 + bi]
        l_true = block + 1
        assert l_true in (L, L - 1)
        if j == L - 2:
            mall[idx] = 1.0 if l_true == L else tril_t
        else:
            mall[idx] = tril_t if l_true == L else 0.0
    # device layout [P(k-local), 16, P(q-local)]
    return np.ascontiguousarray(mall.transpose(1, 0, 2)).astype(NPBF)


def kernel(x, Wq, bq, Wk, bk, Wv, bv, Wp, bp, gamma, beta):
    x = np.asarray(x, np.float32)
    nc = _build_nc()

    WqT = np.ascontiguousarray(np.asarray(Wq, np.float32).T).astype(NPBF)
    WkT = np.ascontiguousarray(np.asarray(Wk, np.float32).T).astype(NPBF)
    WvT = np.ascontiguousarray(np.asarray(Wv, np.float32).T).astype(NPBF)
    WpT = np.ascontiguousarray(np.asarray(Wp, np.float32).T).astype(NPBF)
    bqT = np.ascontiguousarray(np.asarray(bq, np.float32).reshape(EC, P).T)
    bkT = np.ascontiguousarray(np.asarray(bk, np.float32).reshape(EC, P).T)
    cF32 = np.concatenate([bkT, bqT], axis=1)  # [P, 16]
    bcast4 = [
        np.broadcast_to(np.asarray(v, np.float32), (P, E))
        for v in (bv, bp, gamma, beta)
    ]
    # descending padded length = reversed block list
    ownd_map = {0: list(reversed(BLOCKS_A)), 1: list(reversed(BLOCKS_B))}
    cBF_map = {
        hh: np.ascontiguousarray(
            np.concatenate(
                bcast4 + [_make_mall(ownd_map[hh]).reshape(P, 16 * P)], axis=1
            )
        ).astype(NPBF)
        for hh in (0, 1)
    }

    in_maps = []
    for core in range(8):
        b, hh = core // 2, core % 2
        ownd = ownd_map[hh]
        own = np.concatenate([np.arange(blk * P, (blk + 1) * P) for blk in ownd])
        xb = x[b]  # (T, E)
        xT = np.ascontiguousarray(xb.T).astype(NPBF)
        xTq = np.ascontiguousarray(xb[own].T).astype(NPBF)
        in_maps.append(
            {
                "xT": xT,
                "xTq": xTq,
                "WqT": WqT,
                "WkT": WkT,
                "WvT": WvT,
                "WpT": WpT,
                "cF32": cF32,
                "cBF": cBF_map[hh],
            }
        )

    import os

    trace = bool(int(os.environ.get("MHSA_TRACE", "0")))
    res = run_bass_kernel_spmd(
        nc, in_maps, core_ids=list(range(8)), trace=trace,
        trace_cores=list(range(8)) if trace else None,
    )
    if trace and res.exec_time_ns is not None:
        print(f"HW exec time: {res.exec_time_ns} ns")
        if res.mean_exec_time_ns is not None:
            print(f"HW exec mean across cores: {res.mean_exec_time_ns:.0f} ns")
        kernel.last_exec_time_ns = res.exec_time_ns
        kernel.last_trace = res.instructions_and_trace

    out = np.empty((B, T, E), np.float32)
    for core in range(8):
        b, hh = core // 2, core % 2
        ownd = ownd_map[hh]
        y = res.results[core]["y"]  # (NQ, P, E)
        for k, blk in enumerate(ownd):
            out[b, blk * P : (blk + 1) * P, :] = y[k]
    return out


# revision 29
# speedup vs baseline: 1.1225x; 1.0635x over previous
"""Multi-head self-attention (B=4, T=2048, E=1024, H=16) on 8 trn2 NeuronCores.

Sharding: core (b, h) = batch b, token-half h. Each core computes K/V for the
full sequence (duplicated within the batch pair), Q for its own 8 query blocks
of 128 tokens, causal attention for those blocks, then the output projection
and LayerNorm for its own tokens.

Attention restructure (vs the 128-wide-per-head-pair baseline): each core's
query blocks are ordered by DESCENDING padded causal length (16,14,12,10 |
8,6,4,2 key blocks), so for key block j the active query blocks form a
contiguous prefix. Scores/AV run one matmul per (head, group-of-4-q-blocks,
key block) with free dim up to 512, cutting PE instruction count ~3x. The
softmax denominator division runs once per (head, group) on 512 columns.
Projection+LN for the first 4 token blocks is interleaved into the second
(light) attention group to shrink the tail.

Causal balance: query blocks are paired (j, 15-j) so both cores of a batch
process blocks with padded key-lengths 2,4,...,16; host-supplied mask tiles
encode the true causal structure, keeping the compiled program identical
across cores (SPMD).

All matmuls run in bf16 with fp32 PSUM accumulation (validated ~2e-3
scale-relative error vs the fp32 reference).
"""
import json
import numpy as np
import ml_dtypes
from contextlib import ExitStack

import concourse.bass as bass
import concourse.bass_utils as _bass_utils
import concourse.tile as tile
from concourse import mybir
from concourse.bass_utils import run_bass_kernel_spmd

# ----------------------------------------------------------------------------
# Toolchain workarounds for this container's walrus build (see birfix notes):
# 1. EVENT_SEMAPHORE_RANGE_CLEAR InstISA is rejected ("ISA wrong length").
# 2. Engine instructions only carry one semaphore-wait slot; extra waits are
#    peeled onto NoOp carriers on the same engine (order-preserving).
# ----------------------------------------------------------------------------


def _patched_clear_and_free_semaphores(self, sems):
    if not sems:
        return
    sem_nums = [s.num if hasattr(s, "num") else s for s in sems]
    self._state.prepend_free_semaphores(sem_nums)
    for poison_set in self._tile_sem_poison_stack:
        poison_set.update(sem_nums)


def _fix_bir_waits(bir_json: bytes) -> bytes:
    bir = json.loads(bir_json)
    ctr = 0
    changed = False
    for func in bir.get("functions", []):
        for blk in func.get("blocks", []):
            out = []
            for inst in blk.get("instructions", []):
                si = inst.get("sync_info") or {}
                waits = si.get("on_wait") or []
                if len(waits) > 1:
                    for w in waits[:-1]:
                        ctr += 1
                        out.append(
                            {
                                "debug": inst.get("debug"),
                                "engine": inst.get("engine", "SP"),
                                "ins": [],
                                "name": f"IWF-{ctr}",
                                "opcode": "NoOp",
                                "outs": [],
                                "sync_info": {"on_wait": [w]},
                            }
                        )
                    si = dict(si)
                    si["on_wait"] = waits[-1:]
                    inst = dict(inst)
                    inst["sync_info"] = si
                    changed = True
                out.append(inst)
            blk["instructions"] = out
    return json.dumps(bir).encode() if changed else bir_json


_orig_compile_bir_kernel = _bass_utils.compile_bir_kernel


def _patched_compile_bir_kernel(bir_json, tmpdir, neff_name="file.neff"):
    if isinstance(bir_json, str):
        bir_json = bir_json.encode()
    return _orig_compile_bir_kernel(_fix_bir_waits(bir_json), tmpdir, neff_name)


def _install_patches():
    if getattr(bass.Bass, "_mhsa_patched", False):
        return
    bass.Bass.clear_and_free_semaphores = _patched_clear_and_free_semaphores
    bass.Bass._mhsa_patched = True
    _bass_utils.compile_bir_kernel = _patched_compile_bir_kernel
    try:
        import concourse.bass2jax as _b2j

        _b2j.compile_bir_kernel = _patched_compile_bir_kernel
    except ImportError:
        pass


_install_patches()

# ----------------------------------------------------------------------------
# Problem constants (hardcoded per spec)
# ----------------------------------------------------------------------------
B, T, E, H = 4, 2048, 1024, 16
HD = E // H  # 64
P = 128
NB = T // P  # 16 query/key blocks
NQ = 8  # query blocks per core
EC = E // P  # 8 e-chunks
SCALE = 1.0 / float(np.sqrt(T))
EPS = 1e-6
BF = mybir.dt.bfloat16
F32 = mybir.dt.float32
NPBF = ml_dtypes.bfloat16

# query-block assignment: pairs (j, 15-j) so both cores of a batch pair see
# padded lengths {2,4,...,16}; blocks listed in ASCENDING padded length
BLOCKS_A = [0, 2, 4, 6, 9, 11, 13, 15]  # true lengths 1,3,5,7,10,12,14,16
BLOCKS_B = [1, 3, 5, 7, 8, 10, 12, 14]  # true lengths 2,4,6,8,9,11,13,15

# device-side q-block order: DESCENDING padded length; two groups of 4
GLS = {0: (16, 14, 12, 10), 1: (8, 6, 4, 2)}
# mask instances: (group, key block j, boundary q-block index bi); the
# boundary block is always the LAST active block of the prefix at that j
MASK_INST = []
for _g in (0, 1):
    for _j in range(GLS[_g][0]):
        for _bi, _L in enumerate(GLS[_g]):
            if _j in (_L - 2, _L - 1):
                MASK_INST.append((_g, _j, _bi))
MASK_IDX = {(g, j): (idx, bi) for idx, (g, j, bi) in enumerate(MASK_INST)}
assert len(MASK_INST) == 16


def _width(g, j):
    return 128 * sum(1 for L in GLS[g] if L > j)


_nc_cache = {}


def _build_nc():
    if "nc" in _nc_cache:
        return _nc_cache["nc"]
    nc = bass.Bass(num_devices=8)

    # inputs (per-core)
    xT_d = nc.dram_tensor("xT", [E, T], BF, kind="ExternalInput")
    xTq_d = nc.dram_tensor("xTq", [E, NQ * P], BF, kind="ExternalInput")
    WqT_d = nc.dram_tensor("WqT", [E, E], BF, kind="ExternalInput")
    WkT_d = nc.dram_tensor("WkT", [E, E], BF, kind="ExternalInput")
    WvT_d = nc.dram_tensor("WvT", [E, E], BF, kind="ExternalInput")
    WpT_d = nc.dram_tensor("WpT", [E, E], BF, kind="ExternalInput")
    cF32_d = nc.dram_tensor("cF32", [P, 16], F32, kind="ExternalInput")
    cBF_d = nc.dram_tensor("cBF", [P, 4 * E + 16 * P], BF, kind="ExternalInput")
    y_d = nc.dram_tensor("y", [NQ, P, E], F32, kind="ExternalOutput")

    with tile.TileContext(nc) as tc:
        with ExitStack() as ctx:
            consts = ctx.enter_context(tc.tile_pool(name="consts", bufs=1))
            big = ctx.enter_context(tc.tile_pool(name="big", bufs=1))
            wpool = ctx.enter_context(tc.tile_pool(name="wpool", bufs=1))
            # xT is only needed during the QKV phase; its pool is closed
            # before the attention working set is allocated
            xtp = ctx.enter_context(tc.tile_pool(name="xtp", bufs=1))
            _psA_cm = tc.tile_pool(name="psA", bufs=1, space="PSUM")
            ps = _psA_cm.__enter__()

            def load_w(dram, name, interleave_with=None):
                # two half-tiles in a 3-slot rotation: the next projection's
                # first half streams in while the previous one's second half
                # is still being consumed. ONE DMA per half (HWDGE issue is a
                # serialized ~625ns/DMA shared resource — minimize count)
                halves = []
                for hf in range(2):
                    w = wpool.tile(
                        [P, EC, E // 2], BF, tag="wh", bufs=3, name=f"{name}{hf}"
                    )
                    nc.sync.dma_start(
                        w[:, :, :],
                        dram.rearrange("(c p) f -> p c f", p=P)[
                            :, :, hf * 512 : (hf + 1) * 512
                        ],
                    )
                    if interleave_with is not None:
                        interleave_with(hf)
                    halves.append(w)
                return halves

            # PE-critical loads first. HWDGE queue order: Wk half0, xT win0,
            # f32 consts (bk needed by the first bias add), Wk half1, then
            # the remaining xT windows — the ts-outer K loop consumes one
            # window per ~13.7us so the serialized DMA stream stays ahead
            xT = xtp.tile([P, EC, T], BF)
            cF32 = consts.tile([P, 16], F32)
            cBF = consts.tile([P, 4 * E + 16 * P], BF)

            def _xt_w(wi):
                nc.sync.dma_start(
                    xT[:, :, wi * 512 : (wi + 1) * 512],
                    xT_d.rearrange("(c p) t -> p c t", p=P)[
                        :, :, wi * 512 : (wi + 1) * 512
                    ],
                )

            def _wk_companion(hf):
                if hf == 0:
                    _xt_w(0)
                    nc.sync.dma_start(cF32[:, :], cF32_d[:, :])

            Wk = load_w(WkT_d, "Wk", interleave_with=_wk_companion)
            for wi in (1, 2, 3):
                _xt_w(wi)
            xTq = big.tile([P, EC, NQ * P], BF)
            nc.sync.dma_start(
                xTq[:, :, :], xTq_d.rearrange("(c p) t -> p c t", p=P)[:, :, :]
            )
            nc.sync.dma_start(cBF[:, :], cBF_d[:, :])
            # packed-constant layout in cBF: bv | bp | gamma | beta | masks
            OFF_BV, OFF_BP, OFF_G, OFF_B, OFF_M = 0, E, 2 * E, 3 * E, 4 * E

            def mall_at(idx):
                return cBF[:, OFF_M + idx * P : OFF_M + (idx + 1) * P]

            ones64 = consts.tile([P, 64], F32)
            nc.vector.memset(ones64[:], 1.0)

            # persistent intermediates
            KT = big.tile([P, EC, T], BF)  # K^T  [f, t]
            QT = big.tile([P, EC, NQ * P], BF)  # Q^T  [f, t_own]
            Vx = big.tile([P, NB, H, HD + 1], BF)  # V ext [t, h, d|1]
            zT = big.tile([P, EC, NQ * P], BF)  # z^T  [e, t_own]
            nc.vector.memset(Vx[:, :, :, HD : HD + 1], 1.0)

            # ---------------- K^T = Wk^T.T-chunks x xT + bk ----------------
            # ts outer: each xT window feeds all 8 fb groups (~13.7us of PE
            # work) so the next window's DMA completes in the shadow
            for ts_ in range(T // 512):
                for fb in range(EC):
                    pk = ps.tile([P, 512], F32, tag="mm512", bufs=4, name="pk")
                    for c in range(EC):
                        nc.tensor.matmul(
                            pk[:],
                            Wk[fb // 4][:, c, (fb % 4) * P : (fb % 4 + 1) * P],
                            xT[:, c, ts_ * 512 : (ts_ + 1) * 512],
                            start=(c == 0),
                            stop=(c == EC - 1),
                        )
                    nc.vector.tensor_scalar(
                        out=KT[:, fb, ts_ * 512 : (ts_ + 1) * 512],
                        in0=pk[:],
                        scalar1=cF32[:, fb : fb + 1],
                        scalar2=None,
                        op0=mybir.AluOpType.add,
                    )

            # ---------------- V (heads 0-7) = xT x Wv^T[:,0:512] + bv ------
            # V for heads 8-15 and Q feature-blocks 4-7 are deferred: their
            # matmul groups interleave into the exp-bound attention wave for
            # heads 0-7, keeping PE busy while the Activation engine catches
            # up on exponentials.
            Wv = load_w(WvT_d, "Wv")

            def emit_v_group(tb, fs, pool_tag):
                pv = ps.tile([P, 512], F32, tag=pool_tag, bufs=4 if pool_tag == "mm512" else 2, name="pv")
                for c in range(EC):
                    nc.tensor.matmul(
                        pv[:],
                        xT[:, c, tb * P : (tb + 1) * P],
                        Wv[fs][:, c, :],
                        start=(c == 0),
                        stop=(c == EC - 1),
                    )
                nc.vector.tensor_tensor(
                    out=Vx[:, tb, fs * 8 : (fs + 1) * 8, 0:HD],
                    in0=pv[:, :].rearrange("p (h d) -> p h d", d=HD),
                    in1=cBF[:, OFF_BV + fs * 512 : OFF_BV + (fs + 1) * 512].rearrange(
                        "p (h d) -> p h d", d=HD
                    ),
                    op=mybir.AluOpType.add,
                )

            for tb in range(NB):
                emit_v_group(tb, 0, "mm512")

            # ---------------- Q^T (fb 0-3) = Wq^T-chunks x xTq + bq --------
            Wq = load_w(WqT_d, "Wq")

            def emit_q_group(fb, ts_, pool_tag):
                pq = ps.tile([P, 512], F32, tag=pool_tag, bufs=4 if pool_tag == "mm512" else 2, name="pq")
                for c in range(EC):
                    nc.tensor.matmul(
                        pq[:],
                        Wq[fb // 4][:, c, (fb % 4) * P : (fb % 4 + 1) * P],
                        xTq[:, c, ts_ * 512 : (ts_ + 1) * 512],
                        start=(c == 0),
                        stop=(c == EC - 1),
                    )
                nc.vector.tensor_scalar(
                    out=QT[:, fb, ts_ * 512 : (ts_ + 1) * 512],
                    in0=pq[:],
                    scalar1=cF32[:, EC + fb : EC + fb + 1],
                    scalar2=None,
                    op0=mybir.AluOpType.add,
                )

            for fb in range(4):
                for ts_ in range(NQ * P // 512):
                    emit_q_group(fb, ts_, "mm512")

            # Wp streams in during attention (3-slot rotation frees Wq slots)
            Wp = load_w(WpT_d, "Wp")

            # ---------------- attention ----------------
            work = ctx.enter_context(tc.tile_pool(name="work", bufs=2))
            _psA_cm.__exit__(None, None, None)
            _psB_cm = tc.tile_pool(name="psB", bufs=1, space="PSUM")
            ps = _psB_cm.__enter__()

            # unit schedule: wave A = heads 0-7 (g0 then g1), wave B = heads
            # 8-15 g0, residual(0), heads 8-15 g1 with proj tb0-3 interleaved
            units = []
            for g in (0, 1):
                for h in range(8):
                    for p_ in range(GLS[g][0] // 2):
                        units.append((g, h, p_))
            for h in range(8, H):
                for p_ in range(GLS[0][0] // 2):
                    units.append((0, h, p_))
            for h in range(8, H):
                for p_ in range(GLS[1][0] // 2):
                    units.append((1, h, p_))
            WAVE_A_N = 96  # units in wave A
            PART2_AT = WAVE_A_N + 64  # first (g1, h>=8) unit: residual(0) here

            # fillers: deferred V (fs=1) and Q (fb 4-7) groups spread through
            # wave A; projection tb0-3 spread through wave B part 2
            fillers = {}
            deferred = []
            for tb in range(NB):
                deferred.append(("v", tb))
                if tb % 2 == 0:
                    deferred.append(("q", tb // 2))
            for i, d in enumerate(deferred):
                fillers.setdefault(i * WAVE_A_N // len(deferred), []).append(d)
            for i in range(4):
                fillers.setdefault(PART2_AT + 6 + 8 * i, []).append(("proj", i))

            def emit_S(u):
                g, h, p_ = units[u]
                j0 = 2 * p_
                w = _width(g, j0)
                hb = (h % 2) * 64
                pS = ps.tile([P, 1024], F32, tag="pS", bufs=2, name="pS")
                for jj in (0, 1):
                    j = j0 + jj
                    nc.tensor.matmul(
                        pS[:, jj * 512 : jj * 512 + w],
                        KT[hb : hb + 64, h // 2, j * P : (j + 1) * P],
                        QT[hb : hb + 64, h // 2, g * 512 : g * 512 + w],
                        start=True,
                        stop=True,
                    )
                return pS

            def emit_division(h, g, pO):
                hb = (h % 2) * 64
                # reciprocal of the denominators row (accumulated via the
                # Vx ones column), broadcast across 64 partitions with a
                # K=1 matmul into the bank's unused upper rows, one multiply
                rr = work.tile([P, 512], F32, tag="rr", bufs=2, name="rr")
                nc.vector.reciprocal(rr[64:65, :], pO[64:65, :])
                nc.tensor.matmul(
                    pO[64:128, :], ones64[64:65, :], rr[64:65, :],
                    start=True, stop=True,
                )
                # write the normalized head output straight into z^T (saves
                # a serialized HWDGE slot per (head, group))
                nc.vector.tensor_tensor(
                    out=zT[hb : hb + 64, h // 2, g * 512 : (g + 1) * 512],
                    in0=pO[0:HD, :], in1=pO[64:128, :],
                    op=mybir.AluOpType.mult,
                )

            def emit_residual(g):
                cols = slice(g * 512, (g + 1) * 512)
                for c in range(EC):
                    nc.vector.tensor_tensor(
                        out=zT[:, c, cols], in0=zT[:, c, cols],
                        in1=xTq[:, c, cols], op=mybir.AluOpType.add,
                    )

            inv_e = 1.0 / float(E)

            def emit_proj_tb(tb, last=False):
                # bias-add fused with row-sum accumulation (mean), variance
                # via Square(y - mu) with accum, final normalize as one
                # scale+bias activation; gamma/beta on the idle Pool engine
                # except for the last block (shortest critical chain on DVE)
                y_sb = work.tile([P, E], F32, tag="ysb", bufs=2, name="y_sb")
                s0 = work.tile([P, 1], F32, tag="stat", bufs=16, name="s0")
                ysum = work.tile([P, 1], F32, tag="stat", bufs=16, name="ysum")
                for fs in range(E // 512):
                    py = ps.tile([P, 512], F32, tag="py", bufs=2, name="py")
                    for c in range(EC):
                        nc.tensor.matmul(
                            py[:],
                            zT[:, c, tb * P : (tb + 1) * P],
                            Wp[fs][:, c, :],
                            start=(c == 0),
                            stop=(c == EC - 1),
                        )
                    nc.vector.tensor_tensor_reduce(
                        out=y_sb[:, fs * 512 : (fs + 1) * 512],
                        in0=py[:],
                        in1=cBF[:, OFF_BP + fs * 512 : OFF_BP + (fs + 1) * 512],
                        scale=1.0,
                        scalar=(0.0 if fs == 0 else s0[:, 0:1]),
                        op0=mybir.AluOpType.add,
                        op1=mybir.AluOpType.add,
                        accum_out=(s0 if fs == 0 else ysum)[:, 0:1],
                    )
                negmu = work.tile([P, 1], F32, tag="stat", bufs=16, name="negmu")
                nc.vector.tensor_scalar_mul(negmu[:], ysum[:], -inv_e)
                y_c = work.tile([P, E], F32, tag="yc", bufs=2, name="y_c")
                var = work.tile([P, 1], F32, tag="stat", bufs=16, name="var")
                nc.scalar.activation(
                    y_c[:], y_sb[:], mybir.ActivationFunctionType.Square,
                    bias=negmu[:, 0:1], accum_out=var[:],
                )
                rstd = work.tile([P, 1], F32, tag="stat", bufs=16, name="rstd")
                nc.vector.tensor_scalar(
                    out=rstd[:], in0=var[:], scalar1=inv_e, scalar2=float(EPS),
                    op0=mybir.AluOpType.mult, op1=mybir.AluOpType.add,
                )
                nc.scalar.activation(
                    rstd[:], rstd[:], mybir.ActivationFunctionType.Sqrt
                )
                nc.vector.reciprocal(rstd[:], rstd[:])
                nmr = work.tile([P, 1], F32, tag="stat", bufs=16, name="nmr")
                nc.vector.tensor_tensor(
                    out=nmr[:], in0=negmu[:], in1=rstd[:], op=mybir.AluOpType.mult
                )
                nc.scalar.activation(
                    y_c[:], y_sb[:], mybir.ActivationFunctionType.Identity,
                    scale=rstd[:, 0:1], bias=nmr[:, 0:1],
                )
                eng = nc.vector if last else nc.gpsimd
                eng.tensor_tensor(
                    out=y_sb[:], in0=y_c[:], in1=cBF[:, OFF_G : OFF_G + E],
                    op=mybir.AluOpType.mult,
                )
                eng.tensor_tensor(
                    out=y_sb[:], in0=y_sb[:], in1=cBF[:, OFF_B : OFF_B + E],
                    op=mybir.AluOpType.add,
                )
                # issue on the idle Pool queue (SWDGE): the SP queue is
                # blocked behind the Wp loads' semaphore waits mid-attention
                nc.gpsimd.dma_start(y_d[tb, :, :], y_sb[:])

            pO_cur = None
            pending_div = None
            prev_S = emit_S(0)
            for u, (g, h, p_) in enumerate(units):
                j0 = 2 * p_
                w = _width(g, j0)
                maxL = GLS[g][0]
                if p_ == 0:
                    pO_cur = ps.tile([P, 512], F32, tag="pO", bufs=2, name="pO")
                pO = pO_cur
                pS = prev_S
                eS = work.tile([P, 1024], BF, tag="eS", bufs=3, name="eS")
                nc.scalar.activation(
                    eS[:, :].rearrange("p (u q) -> p u q", u=2)[:, :, 0:w],
                    pS[:, :].rearrange("p (u q) -> p u q", u=2)[:, :, 0:w],
                    mybir.ActivationFunctionType.Exp,
                    scale=SCALE,
                )
                if u + 1 < len(units):
                    prev_S = emit_S(u + 1)
                if pending_div is not None and p_ == 0:
                    pending_div()
                    pending_div = None
                if u == PART2_AT:
                    emit_residual(0)
                for kind, arg in fillers.get(u, ()):
                    if kind == "v":
                        emit_v_group(arg, 1, "py")
                    elif kind == "q":
                        emit_q_group(4 + arg // 2, arg % 2, "py")
                    else:
                        emit_proj_tb(arg)
                for jj in (0, 1):
                    j = j0 + jj
                    mi = MASK_IDX.get((g, j))
                    if mi is not None:
                        idx, bi = mi
                        cs = slice(jj * 512 + bi * P, jj * 512 + (bi + 1) * P)
                        nc.vector.tensor_tensor(
                            out=eS[:, cs], in0=eS[:, cs],
                            in1=mall_at(idx), op=mybir.AluOpType.mult,
                        )
                    nc.tensor.matmul(
                        pO[0 : HD + 1, 0:w],
                        Vx[:, j, h, :],
                        eS[:, jj * 512 : jj * 512 + w],
                        start=(j == 0),
                        stop=(j == maxL - 1),
                        skip_group_check=True,
                    )
                if j0 + 1 == maxL - 1:

                    def _div(h=h, g=g, pO=pO):
                        emit_division(h, g, pO)

                    pending_div = _div
            if pending_div is not None:
                pending_div()
                pending_div = None

            # ---------------- tail: residual + projection for group 1 ------
            emit_residual(1)
            for tb in range(4, NQ):
                emit_proj_tb(tb, last=(tb == NQ - 1))

            _psB_cm.__exit__(None, None, None)

    _nc_cache["nc"] = nc
    return nc


def _make_mall(ownd):
    """Mask tiles for this core's descending-ordered q-blocks.

    Instance (g, j, bi): multiply eS columns of boundary q-block bi at key
    block j. Pattern depends on whether the block's true length equals the
    padded length (l_true == L) or falls one short (l_true == L-1)."""
    tril_t = (np.arange(P)[:, None] <= np.arange(P)[None, :]).astype(np.float32)
    mall = np.zeros((16, P, P), np.float32)
    for idx, (g, j, bi) in enumerate(MASK_INST):
        L = GLS[g][bi]
        block = ownd[g * 4 + bi]
        l_true = block + 1
        assert l_true in (L, L - 1)
        if j == L - 2:
            mall[idx] = 1.0 if l_true == L else tril_t
        else:
            mall[idx] = tril_t if l_true == L else 0.0
    # device layout [P(k-local), 16, P(q-local)]
    return np.ascontiguousarray(mall.transpose(1, 0, 2)).astype(NPBF)


def kernel(x, Wq, bq, Wk, bk, Wv, bv, Wp, bp, gamma, beta):
    x = np.asarray(x, np.float32)
    nc = _build_nc()

    WqT = np.ascontiguousarray(np.asarray(Wq, np.float32).T).astype(NPBF)
    WkT = np.ascontiguousarray(np.asarray(Wk, np.float32).T).astype(NPBF)
    WvT = np.ascontiguousarray(np.asarray(Wv, np.float32).T).astype(NPBF)
    WpT = np.ascontiguousarray(np.asarray(Wp, np.float32).T).astype(NPBF)
    bqT = np.ascontiguousarray(np.asarray(bq, np.float32).reshape(EC, P).T)
    bkT = np.ascontiguousarray(np.asarray(bk, np.float32).reshape(EC, P).T)
    cF32 = np.concatenate([bkT, bqT], axis=1)  # [P, 16]
    bcast4 = [
        np.broadcast_to(np.asarray(v, np.float32), (P, E))
        for v in (bv, bp, gamma, beta)
    ]
    # descending padded length = reversed block list
    ownd_map = {0: list(reversed(BLOCKS_A)), 1: list(reversed(BLOCKS_B))}
    cBF_map = {
        hh: np.ascontiguousarray(
            np.concatenate(
                bcast4 + [_make_mall(ownd_map[hh]).reshape(P, 16 * P)], axis=1
            )
        ).astype(NPBF)
        for hh in (0, 1)
    }

    in_maps = []
    for core in range(8):
        b, hh = core // 2, core % 2
        ownd = ownd_map[hh]
        own = np.concatenate([np.arange(blk * P, (blk + 1) * P) for blk in ownd])
        xb = x[b]  # (T, E)
        xT = np.ascontiguousarray(xb.T).astype(NPBF)
        xTq = np.ascontiguousarray(xb[own].T).astype(NPBF)
        in_maps.append(
            {
                "xT": xT,
                "xTq": xTq,
                "WqT": WqT,
                "WkT": WkT,
                "WvT": WvT,
                "WpT": WpT,
                "cF32": cF32,
                "cBF": cBF_map[hh],
            }
        )

    import os

    trace = bool(int(os.environ.get("MHSA_TRACE", "0")))
    res = run_bass_kernel_spmd(
        nc, in_maps, core_ids=list(range(8)), trace=trace,
        trace_cores=list(range(8)) if trace else None,
    )
    if trace and res.exec_time_ns is not None:
        print(f"HW exec time: {res.exec_time_ns} ns")
        if res.mean_exec_time_ns is not None:
            print(f"HW exec mean across cores: {res.mean_exec_time_ns:.0f} ns")
        kernel.last_exec_time_ns = res.exec_time_ns
        kernel.last_trace = res.instructions_and_trace

    out = np.empty((B, T, E), np.float32)
    for core in range(8):
        b, hh = core // 2, core % 2
        ownd = ownd_map[hh]
        y = res.results[core]["y"]  # (NQ, P, E)
        for k, blk in enumerate(ownd):
            out[b, blk * P : (blk + 1) * P, :] = y[k]
    return out


# revision 34
# speedup vs baseline: 1.1243x; 1.0016x over previous
"""Multi-head self-attention (B=4, T=2048, E=1024, H=16) on 8 trn2 NeuronCores.

Sharding: core (b, h) = batch b, token-half h. Each core computes K/V for the
full sequence (duplicated within the batch pair), Q for its own 8 query blocks
of 128 tokens, causal attention for those blocks, then the output projection
and LayerNorm for its own tokens.

Attention restructure (vs the 128-wide-per-head-pair baseline): each core's
query blocks are ordered by DESCENDING padded causal length (16,14,12,10 |
8,6,4,2 key blocks), so for key block j the active query blocks form a
contiguous prefix. Scores/AV run one matmul per (head, group-of-4-q-blocks,
key block) with free dim up to 512, cutting PE instruction count ~3x. The
softmax denominator division runs once per (head, group) on 512 columns.
Projection+LN for the first 4 token blocks is interleaved into the second
(light) attention group to shrink the tail.

Causal balance: query blocks are paired (j, 15-j) so both cores of a batch
process blocks with padded key-lengths 2,4,...,16; host-supplied mask tiles
encode the true causal structure, keeping the compiled program identical
across cores (SPMD).

All matmuls run in bf16 with fp32 PSUM accumulation (validated ~2e-3
scale-relative error vs the fp32 reference).
"""
import json
import numpy as np
import ml_dtypes
from contextlib import ExitStack

import concourse.bass as bass
import concourse.bass_utils as _bass_utils
import concourse.tile as tile
from concourse import mybir
from concourse.bass_utils import run_bass_kernel_spmd

# ----------------------------------------------------------------------------
# Toolchain workarounds for this container's walrus build (see birfix notes):
# 1. EVENT_SEMAPHORE_RANGE_CLEAR InstISA is rejected ("ISA wrong length").
# 2. Engine instructions only carry one semaphore-wait slot; extra waits are
#    peeled onto NoOp carriers on the same engine (order-preserving).
# ----------------------------------------------------------------------------


def _patched_clear_and_free_semaphores(self, sems):
    if not sems:
        return
    sem_nums = [s.num if hasattr(s, "num") else s for s in sems]
    self._state.prepend_free_semaphores(sem_nums)
    for poison_set in self._tile_sem_poison_stack:
        poison_set.update(sem_nums)


def _fix_bir_waits(bir_json: bytes) -> bytes:
    bir = json.loads(bir_json)
    ctr = 0
    changed = False
    for func in bir.get("functions", []):
        for blk in func.get("blocks", []):
            out = []
            for inst in blk.get("instructions", []):
                si = inst.get("sync_info") or {}
                waits = si.get("on_wait") or []
                if len(waits) > 1:
                    for w in waits[:-1]:
                        ctr += 1
                        out.append(
                            {
                                "debug": inst.get("debug"),
                                "engine": inst.get("engine", "SP"),
                                "ins": [],
                                "name": f"IWF-{ctr}",
                                "opcode": "NoOp",
                                "outs": [],
                                "sync_info": {"on_wait": [w]},
                            }
                        )
                    si = dict(si)
                    si["on_wait"] = waits[-1:]
                    inst = dict(inst)
                    inst["sync_info"] = si
                    changed = True
                out.append(inst)
            blk["instructions"] = out
    return json.dumps(bir).encode() if changed else bir_json


_orig_compile_bir_kernel = _bass_utils.compile_bir_kernel


def _patched_compile_bir_kernel(bir_json, tmpdir, neff_name="file.neff"):
    if isinstance(bir_json, str):
        bir_json = bir_json.encode()
    return _orig_compile_bir_kernel(_fix_bir_waits(bir_json), tmpdir, neff_name)


def _install_patches():
    if getattr(bass.Bass, "_mhsa_patched", False):
        return
    bass.Bass.clear_and_free_semaphores = _patched_clear_and_free_semaphores
    bass.Bass._mhsa_patched = True
    _bass_utils.compile_bir_kernel = _patched_compile_bir_kernel
    try:
        import concourse.bass2jax as _b2j

        _b2j.compile_bir_kernel = _patched_compile_bir_kernel
    except ImportError:
        pass


_install_patches()

# ----------------------------------------------------------------------------
# Problem constants (hardcoded per spec)
# ----------------------------------------------------------------------------
B, T, E, H = 4, 2048, 1024, 16
HD = E // H  # 64
P = 128
NB = T // P  # 16 query/key blocks
NQ = 8  # query blocks per core
EC = E // P  # 8 e-chunks
SCALE = 1.0 / float(np.sqrt(T))
EPS = 1e-6
BF = mybir.dt.bfloat16
F32 = mybir.dt.float32
NPBF = ml_dtypes.bfloat16

# query-block assignment: pairs (j, 15-j) so both cores of a batch pair see
# padded lengths {2,4,...,16}; blocks listed in ASCENDING padded length
BLOCKS_A = [0, 2, 4, 6, 9, 11, 13, 15]  # true lengths 1,3,5,7,10,12,14,16
BLOCKS_B = [1, 3, 5, 7, 8, 10, 12, 14]  # true lengths 2,4,6,8,9,11,13,15

# device-side q-block order: DESCENDING padded length; two groups of 4
GLS = {0: (16, 14, 12, 10), 1: (8, 6, 4, 2)}
# mask instances: (group, key block j, boundary q-block index bi); the
# boundary block is always the LAST active block of the prefix at that j
MASK_INST = []
for _g in (0, 1):
    for _j in range(GLS[_g][0]):
        for _bi, _L in enumerate(GLS[_g]):
            if _j in (_L - 2, _L - 1):
                MASK_INST.append((_g, _j, _bi))
MASK_IDX = {(g, j): (idx, bi) for idx, (g, j, bi) in enumerate(MASK_INST)}
assert len(MASK_INST) == 16


def _width(g, j):
    return 128 * sum(1 for L in GLS[g] if L > j)


_nc_cache = {}


def _build_nc():
    if "nc" in _nc_cache:
        return _nc_cache["nc"]
    nc = bass.Bass(num_devices=8)

    # inputs (per-core)
    xT_d = nc.dram_tensor("xT", [E, T], BF, kind="ExternalInput")
    xTq_d = nc.dram_tensor("xTq", [E, NQ * P], BF, kind="ExternalInput")
    WqT_d = nc.dram_tensor("WqT", [E, E], BF, kind="ExternalInput")
    WkT_d = nc.dram_tensor("WkT", [E, E], BF, kind="ExternalInput")
    WvT_d = nc.dram_tensor("WvT", [E, E], BF, kind="ExternalInput")
    WpT_d = nc.dram_tensor("WpT", [E, E], BF, kind="ExternalInput")
    cF32_d = nc.dram_tensor("cF32", [P, 16], F32, kind="ExternalInput")
    cBF_d = nc.dram_tensor("cBF", [P, 4 * E + 16 * P], BF, kind="ExternalInput")
    y_d = nc.dram_tensor("y", [NQ, P, E], F32, kind="ExternalOutput")

    with tile.TileContext(nc) as tc:
        with ExitStack() as ctx:
            consts = ctx.enter_context(tc.tile_pool(name="consts", bufs=1))
            big = ctx.enter_context(tc.tile_pool(name="big", bufs=1))
            wpool = ctx.enter_context(tc.tile_pool(name="wpool", bufs=1))
            # xT is only needed during the QKV phase; its pool is closed
            # before the attention working set is allocated
            xtp = ctx.enter_context(tc.tile_pool(name="xtp", bufs=1))
            _psA_cm = tc.tile_pool(name="psA", bufs=1, space="PSUM")
            ps = _psA_cm.__enter__()

            def load_w(dram, name, interleave_with=None):
                # two half-tiles in a 3-slot rotation: the next projection's
                # first half streams in while the previous one's second half
                # is still being consumed. ONE DMA per half (HWDGE issue is a
                # serialized ~625ns/DMA shared resource — minimize count)
                halves = []
                for hf in range(2):
                    w = wpool.tile(
                        [P, EC, E // 2], BF, tag="wh", bufs=3, name=f"{name}{hf}"
                    )
                    nc.sync.dma_start(
                        w[:, :, :],
                        dram.rearrange("(c p) f -> p c f", p=P)[
                            :, :, hf * 512 : (hf + 1) * 512
                        ],
                    )
                    if interleave_with is not None:
                        interleave_with(hf)
                    halves.append(w)
                return halves

            # PE-critical loads first. HWDGE queue order: Wk half0, xT win0,
            # f32 consts (bk needed by the first bias add), Wk half1, then
            # the remaining xT windows — the ts-outer K loop consumes one
            # window per ~13.7us so the serialized DMA stream stays ahead
            xT = xtp.tile([P, EC, T], BF)
            cF32 = consts.tile([P, 16], F32)
            cBF = consts.tile([P, 4 * E + 16 * P], BF)

            def _xt_w(wi):
                nc.sync.dma_start(
                    xT[:, :, wi * 512 : (wi + 1) * 512],
                    xT_d.rearrange("(c p) t -> p c t", p=P)[
                        :, :, wi * 512 : (wi + 1) * 512
                    ],
                )

            def _wk_companion(hf):
                if hf == 0:
                    _xt_w(0)
                    nc.sync.dma_start(cF32[:, :], cF32_d[:, :])

            Wk = load_w(WkT_d, "Wk", interleave_with=_wk_companion)
            for wi in (1, 2, 3):
                _xt_w(wi)
            xTq = big.tile([P, EC, NQ * P], BF)
            nc.sync.dma_start(
                xTq[:, :, :], xTq_d.rearrange("(c p) t -> p c t", p=P)[:, :, :]
            )
            nc.sync.dma_start(cBF[:, :], cBF_d[:, :])
            # packed-constant layout in cBF: bv | bp | gamma | beta | masks
            OFF_BV, OFF_BP, OFF_G, OFF_B, OFF_M = 0, E, 2 * E, 3 * E, 4 * E

            def mall_at(idx):
                return cBF[:, OFF_M + idx * P : OFF_M + (idx + 1) * P]

            ones64 = consts.tile([P, 64], F32)
            nc.vector.memset(ones64[:], 1.0)

            # persistent intermediates
            KT = big.tile([P, EC, T], BF)  # K^T  [f, t]
            QT = big.tile([P, EC, NQ * P], BF)  # Q^T  [f, t_own]
            Vx = big.tile([P, NB, H, HD + 1], BF)  # V ext [t, h, d|1]
            zT = big.tile([P, EC, NQ * P], BF)  # z^T  [e, t_own]
            nc.vector.memset(Vx[:, :, :, HD : HD + 1], 1.0)

            # ---------------- K^T = Wk^T.T-chunks x xT + bk ----------------
            # ts outer: each xT window feeds all 8 fb groups (~13.7us of PE
            # work) so the next window's DMA completes in the shadow
            for ts_ in range(T // 512):
                for fb in range(EC):
                    pk = ps.tile([P, 512], F32, tag="mm512", bufs=4, name="pk")
                    for c in range(EC):
                        nc.tensor.matmul(
                            pk[:],
                            Wk[fb // 4][:, c, (fb % 4) * P : (fb % 4 + 1) * P],
                            xT[:, c, ts_ * 512 : (ts_ + 1) * 512],
                            start=(c == 0),
                            stop=(c == EC - 1),
                        )
                    nc.vector.tensor_scalar(
                        out=KT[:, fb, ts_ * 512 : (ts_ + 1) * 512],
                        in0=pk[:],
                        scalar1=cF32[:, fb : fb + 1],
                        scalar2=None,
                        op0=mybir.AluOpType.add,
                    )

            # ---------------- V (heads 0-7) = xT x Wv^T[:,0:512] + bv ------
            # V for heads 8-15 and Q feature-blocks 4-7 are deferred: their
            # matmul groups interleave into the exp-bound attention wave for
            # heads 0-7, keeping PE busy while the Activation engine catches
            # up on exponentials.
            Wv = load_w(WvT_d, "Wv")

            def emit_v_group(tb, fs, pool_tag):
                pv = ps.tile([P, 512], F32, tag=pool_tag, bufs=4 if pool_tag == "mm512" else 2, name="pv")
                for c in range(EC):
                    nc.tensor.matmul(
                        pv[:],
                        xT[:, c, tb * P : (tb + 1) * P],
                        Wv[fs][:, c, :],
                        start=(c == 0),
                        stop=(c == EC - 1),
                    )
                nc.vector.tensor_tensor(
                    out=Vx[:, tb, fs * 8 : (fs + 1) * 8, 0:HD],
                    in0=pv[:, :].rearrange("p (h d) -> p h d", d=HD),
                    in1=cBF[:, OFF_BV + fs * 512 : OFF_BV + (fs + 1) * 512].rearrange(
                        "p (h d) -> p h d", d=HD
                    ),
                    op=mybir.AluOpType.add,
                )

            for tb in range(NB):
                emit_v_group(tb, 0, "mm512")

            # ---------------- Q^T (fb 0-3) = Wq^T-chunks x xTq + bq --------
            Wq = load_w(WqT_d, "Wq")

            def emit_q_group(fb, ts_, pool_tag):
                pq = ps.tile([P, 512], F32, tag=pool_tag, bufs=4 if pool_tag == "mm512" else 2, name="pq")
                for c in range(EC):
                    nc.tensor.matmul(
                        pq[:],
                        Wq[fb // 4][:, c, (fb % 4) * P : (fb % 4 + 1) * P],
                        xTq[:, c, ts_ * 512 : (ts_ + 1) * 512],
                        start=(c == 0),
                        stop=(c == EC - 1),
                    )
                nc.vector.tensor_scalar(
                    out=QT[:, fb, ts_ * 512 : (ts_ + 1) * 512],
                    in0=pq[:],
                    scalar1=cF32[:, EC + fb : EC + fb + 1],
                    scalar2=None,
                    op0=mybir.AluOpType.add,
                )

            for fb in range(4):
                for ts_ in range(NQ * P // 512):
                    emit_q_group(fb, ts_, "mm512")

            # Wp streams in during attention (3-slot rotation frees Wq slots)
            Wp = load_w(WpT_d, "Wp")

            # ---------------- attention ----------------
            work = ctx.enter_context(tc.tile_pool(name="work", bufs=2))
            _psA_cm.__exit__(None, None, None)
            _psB_cm = tc.tile_pool(name="psB", bufs=1, space="PSUM")
            ps = _psB_cm.__enter__()

            # unit schedule: wave A = heads 0-7 (g0 then g1), wave B = heads
            # 8-15 g0, residual(0), heads 8-15 g1 with proj tb0-3 interleaved
            units = []
            for g in (0, 1):
                for h in range(8):
                    for p_ in range(GLS[g][0] // 2):
                        units.append((g, h, p_))
            for h in range(8, H):
                for p_ in range(GLS[0][0] // 2):
                    units.append((0, h, p_))
            for h in range(8, H):
                for p_ in range(GLS[1][0] // 2):
                    units.append((1, h, p_))
            WAVE_A_N = 96  # units in wave A
            PART2_AT = WAVE_A_N + 64  # first (g1, h>=8) unit: residual(0) here

            # fillers: deferred V (fs=1) and Q (fb 4-7) groups spread through
            # wave A (Q first — needed by wave B's first scores) and slightly
            # into wave B part 1; projection tb0-3 spread through part 2
            fillers = {}
            deferred = []
            for tb in range(NB):
                deferred.append(("v", tb))
                if tb < 8:
                    deferred.append(("q", tb))
            for i, d in enumerate(deferred):
                fillers.setdefault(i * (WAVE_A_N + 5) // len(deferred), []).append(d)
            for i in range(4):
                fillers.setdefault(PART2_AT + 6 + 8 * i, []).append(("proj", i))

            def emit_S(u):
                g, h, p_ = units[u]
                j0 = 2 * p_
                w = _width(g, j0)
                hb = (h % 2) * 64
                pS = ps.tile([P, 1024], F32, tag="pS", bufs=2, name="pS")
                for jj in (0, 1):
                    j = j0 + jj
                    nc.tensor.matmul(
                        pS[:, jj * 512 : jj * 512 + w],
                        KT[hb : hb + 64, h // 2, j * P : (j + 1) * P],
                        QT[hb : hb + 64, h // 2, g * 512 : g * 512 + w],
                        start=True,
                        stop=True,
                    )
                return pS

            def emit_division(h, g, pO):
                hb = (h % 2) * 64
                # reciprocal of the denominators row (accumulated via the
                # Vx ones column), broadcast across 64 partitions with a
                # K=1 matmul into the bank's unused upper rows, one multiply
                rr = work.tile([P, 512], F32, tag="rr", bufs=2, name="rr")
                nc.vector.reciprocal(rr[64:65, :], pO[64:65, :])
                nc.tensor.matmul(
                    pO[64:128, :], ones64[64:65, :], rr[64:65, :],
                    start=True, stop=True,
                )
                # write the normalized head output straight into z^T (saves
                # a serialized HWDGE slot per (head, group))
                nc.vector.tensor_tensor(
                    out=zT[hb : hb + 64, h // 2, g * 512 : (g + 1) * 512],
                    in0=pO[0:HD, :], in1=pO[64:128, :],
                    op=mybir.AluOpType.mult,
                )

            def emit_residual(g, chunks):
                cols = slice(g * 512, (g + 1) * 512)
                for c in chunks:
                    nc.vector.tensor_tensor(
                        out=zT[:, c, cols], in0=zT[:, c, cols],
                        in1=xTq[:, c, cols], op=mybir.AluOpType.add,
                    )

            inv_e = 1.0 / float(E)

            def emit_proj_tb(tb, last=False):
                # bias-add fused with row-sum accumulation (mean), variance
                # via Square(y - mu) with accum, final normalize as one
                # scale+bias activation; gamma/beta on the idle Pool engine
                # except for the last block (shortest critical chain on DVE)
                y_sb = work.tile([P, E], F32, tag="ysb", bufs=2, name="y_sb")
                s0 = work.tile([P, 1], F32, tag="stat", bufs=16, name="s0")
                ysum = work.tile([P, 1], F32, tag="stat", bufs=16, name="ysum")
                for fs in range(E // 512):
                    py = ps.tile([P, 512], F32, tag="py", bufs=2, name="py")
                    for c in range(EC):
                        nc.tensor.matmul(
                            py[:],
                            zT[:, c, tb * P : (tb + 1) * P],
                            Wp[fs][:, c, :],
                            start=(c == 0),
                            stop=(c == EC - 1),
                        )
                    nc.vector.tensor_tensor_reduce(
                        out=y_sb[:, fs * 512 : (fs + 1) * 512],
                        in0=py[:],
                        in1=cBF[:, OFF_BP + fs * 512 : OFF_BP + (fs + 1) * 512],
                        scale=1.0,
                        scalar=(0.0 if fs == 0 else s0[:, 0:1]),
                        op0=mybir.AluOpType.add,
                        op1=mybir.AluOpType.add,
                        accum_out=(s0 if fs == 0 else ysum)[:, 0:1],
                    )
                negmu = work.tile([P, 1], F32, tag="stat", bufs=16, name="negmu")
                nc.vector.tensor_scalar_mul(negmu[:], ysum[:], -inv_e)
                y_c = work.tile([P, E], F32, tag="yc", bufs=2, name="y_c")
                var = work.tile([P, 1], F32, tag="stat", bufs=16, name="var")
                nc.scalar.activation(
                    y_c[:], y_sb[:], mybir.ActivationFunctionType.Square,
                    bias=negmu[:, 0:1], accum_out=var[:],
                )
                rstd = work.tile([P, 1], F32, tag="stat", bufs=16, name="rstd")
                nc.vector.tensor_scalar(
                    out=rstd[:], in0=var[:], scalar1=inv_e, scalar2=float(EPS),
                    op0=mybir.AluOpType.mult, op1=mybir.AluOpType.add,
                )
                nc.scalar.activation(
                    rstd[:], rstd[:], mybir.ActivationFunctionType.Sqrt
                )
                nc.vector.reciprocal(rstd[:], rstd[:])
                nmr = work.tile([P, 1], F32, tag="stat", bufs=16, name="nmr")
                nc.vector.tensor_tensor(
                    out=nmr[:], in0=negmu[:], in1=rstd[:], op=mybir.AluOpType.mult
                )
                nc.scalar.activation(
                    y_c[:], y_sb[:], mybir.ActivationFunctionType.Identity,
                    scale=rstd[:, 0:1], bias=nmr[:, 0:1],
                )
                eng = nc.vector if last else nc.gpsimd
                eng.tensor_tensor(
                    out=y_sb[:], in0=y_c[:], in1=cBF[:, OFF_G : OFF_G + E],
                    op=mybir.AluOpType.mult,
                )
                eng.tensor_tensor(
                    out=y_sb[:], in0=y_sb[:], in1=cBF[:, OFF_B : OFF_B + E],
                    op=mybir.AluOpType.add,
                )
                # issue on the idle Pool queue (SWDGE): the SP queue is
                # blocked behind the Wp loads' semaphore waits mid-attention
                nc.gpsimd.dma_start(y_d[tb, :, :], y_sb[:])

            pO_cur = None
            pending_div = None
            prev_S = emit_S(0)
            for u, (g, h, p_) in enumerate(units):
                j0 = 2 * p_
                w = _width(g, j0)
                maxL = GLS[g][0]
                if p_ == 0:
                    pO_cur = ps.tile([P, 512], F32, tag="pO", bufs=2, name="pO")
                pO = pO_cur
                pS = prev_S
                eS = work.tile([P, 1024], BF, tag="eS", bufs=3, name="eS")
                nc.scalar.activation(
                    eS[:, :].rearrange("p (u q) -> p u q", u=2)[:, :, 0:w],
                    pS[:, :].rearrange("p (u q) -> p u q", u=2)[:, :, 0:w],
                    mybir.ActivationFunctionType.Exp,
                    scale=SCALE,
                )
                if u + 1 < len(units):
                    prev_S = emit_S(u + 1)
                if pending_div is not None and p_ == 0:
                    pending_div()
                    pending_div = None
                if u == PART2_AT:
                    # all g0 divisions and heads 0-7's g1 divisions are done
                    emit_residual(0, range(EC))
                    emit_residual(1, range(4))
                for kind, arg in fillers.get(u, ()):
                    if kind == "v":
                        emit_v_group(arg, 1, "py")
                    elif kind == "q":
                        emit_q_group(4 + arg // 2, arg % 2, "py")
                    else:
                        emit_proj_tb(arg)
                for jj in (0, 1):
                    j = j0 + jj
                    mi = MASK_IDX.get((g, j))
                    if mi is not None:
                        idx, bi = mi
                        cs = slice(jj * 512 + bi * P, jj * 512 + (bi + 1) * P)
                        nc.vector.tensor_tensor(
                            out=eS[:, cs], in0=eS[:, cs],
                            in1=mall_at(idx), op=mybir.AluOpType.mult,
                        )
                    nc.tensor.matmul(
                        pO[0 : HD + 1, 0:w],
                        Vx[:, j, h, :],
                        eS[:, jj * 512 : jj * 512 + w],
                        start=(j == 0),
                        stop=(j == maxL - 1),
                        skip_group_check=True,
                    )
                if j0 + 1 == maxL - 1:

                    def _div(h=h, g=g, pO=pO):
                        emit_division(h, g, pO)
                        if g == 1 and h >= 9 and h % 2 == 1:
                            # z^T feature chunk h//2 complete for both column
                            # groups: add the residual now so the tail
                            # projection's contraction can start early
                            emit_residual(1, [h // 2])

                    pending_div = _div
            if pending_div is not None:
                pending_div()
                pending_div = None

            # ---------------- tail: residual + projection for group 1 ------
            for tb in range(4, NQ):
                emit_proj_tb(tb, last=(tb == NQ - 1))

            _psB_cm.__exit__(None, None, None)

    _nc_cache["nc"] = nc
    return nc


def _make_mall(ownd):
    """Mask tiles for this core's descending-ordered q-blocks.

    Instance (g, j, bi): multiply eS columns of boundary q-block bi at key
    block j. Pattern depends on whether the block's true length equals the
    padded length (l_true == L) or falls one short (l_true == L-1)."""
    tril_t = (np.arange(P)[:, None] <= np.arange(P)[None, :]).astype(np.float32)
    mall = np.zeros((16, P, P), np.float32)
    for idx, (g, j, bi) in enumerate(MASK_INST):
        L = GLS[g][bi]
        block = ownd[g * 4 + bi]
        l_true = block + 1
        assert l_true in (L, L - 1)
        if j == L - 2:
            mall[idx] = 1.0 if l_true == L else tril_t
        else:
            mall[idx] = tril_t if l_true == L else 0.0
    # device layout [P(k-local), 16, P(q-local)]
    return np.ascontiguousarray(mall.transpose(1, 0, 2)).astype(NPBF)


def kernel(x, Wq, bq, Wk, bk, Wv, bv, Wp, bp, gamma, beta):
    x = np.asarray(x, np.float32)
    nc = _build_nc()

    WqT = np.ascontiguousarray(np.asarray(Wq, np.float32).T).astype(NPBF)
    WkT = np.ascontiguousarray(np.asarray(Wk, np.float32).T).astype(NPBF)
    WvT = np.ascontiguousarray(np.asarray(Wv, np.float32).T).astype(NPBF)
    WpT = np.ascontiguousarray(np.asarray(Wp, np.float32).T).astype(NPBF)
    bqT = np.ascontiguousarray(np.asarray(bq, np.float32).reshape(EC, P).T)
    bkT = np.ascontiguousarray(np.asarray(bk, np.float32).reshape(EC, P).T)
    cF32 = np.concatenate([bkT, bqT], axis=1)  # [P, 16]
    bcast4 = [
        np.broadcast_to(np.asarray(v, np.float32), (P, E))
        for v in (bv, bp, gamma, beta)
    ]
    # descending padded length = reversed block list
    ownd_map = {0: list(reversed(BLOCKS_A)), 1: list(reversed(BLOCKS_B))}
    cBF_map = {
        hh: np.ascontiguousarray(
            np.concatenate(
                bcast4 + [_make_mall(ownd_map[hh]).reshape(P, 16 * P)], axis=1
            )
        ).astype(NPBF)
        for hh in (0, 1)
    }

    in_maps = []
    for core in range(8):
        b, hh = core // 2, core % 2
        ownd = ownd_map[hh]
        own = np.concatenate([np.arange(blk * P, (blk + 1) * P) for blk in ownd])
        xb = x[b]  # (T, E)
        xT = np.ascontiguousarray(xb.T).astype(NPBF)
        xTq = np.ascontiguousarray(xb[own].T).astype(NPBF)
        in_maps.append(
            {
                "xT": xT,
                "xTq": xTq,
                "WqT": WqT,
                "WkT": WkT,
                "WvT": WvT,
                "WpT": WpT,
                "cF32": cF32,
                "cBF": cBF_map[hh],
            }
        )

    import os

    trace = bool(int(os.environ.get("MHSA_TRACE", "0")))
    res = run_bass_kernel_spmd(
        nc, in_maps, core_ids=list(range(8)), trace=trace,
        trace_cores=list(range(8)) if trace else None,
    )
    if trace and res.exec_time_ns is not None:
        print(f"HW exec time: {res.exec_time_ns} ns")
        if res.mean_exec_time_ns is not None:
            print(f"HW exec mean across cores: {res.mean_exec_time_ns:.0f} ns")
        kernel.last_exec_time_ns = res.exec_time_ns
        kernel.last_trace = res.instructions_and_trace

    out = np.empty((B, T, E), np.float32)
    for core in range(8):
        b, hh = core // 2, core % 2
        ownd = ownd_map[hh]
        y = res.results[core]["y"]  # (NQ, P, E)
        for k, blk in enumerate(ownd):
            out[b, blk * P : (blk + 1) * P, :] = y[k]
    return out
